# revision 6
# baseline (speedup 1.0000x reference)
"""Causal self-attention (B=2, T=2048, C=1024, H=16, D=64) on 8 trn2 cores.

Sharding: core c -> batch b = c // 4, head-group g = c % 4 (4 heads each).
Data-parallel over B, tensor-parallel (Megatron) over heads for the
qkv / proj linears. Each core computes its head-group's attention and a
partial output projection; the host sums the 4 partials per batch and
adds the proj bias.

Structure (v1, from the ~161us baseline):
  * Inputs batched into ~11 DMA descriptors (descriptor issue on the Sync
    engine costs ~600ns each; the old ~30 descriptors serialized the
    startup). x is loaded ONCE in bf16; the fp8 copy for the q/k path is
    cast on-chip by the DVE (saves 2.1MB of HBM traffic on the critical
    startup path).
  * q/k projection (fp8 DoubleRow, weights pre-scaled x64 on host) runs
    kp-OUTER across 8 concurrent PSUM accumulation groups so the PE
    tracks the x DMA stream as chunk-pairs land.
  * v projection: first 4 row-chunks serial (k-inner), rest moved into
    the attention foreign queue.
  * Attention as 2 head-PAIRS, pt/et tiles laid out [128, 2, 512]
    (h0/h1 blocks); ONE exp ACTIVATE covers both heads via a 3D AP.
  * Causal mask: DVE multiply of the exp output's 128-wide diag strip by
    a constant 0/1 triangular [128, 2, 128] mask - off the PE entirely
    (the old identity-matmul mask cost ~417ns of LDW+MM per diag unit),
    hidden in the exp->AV lag slack.
  * Softmax denominator from a ones-column in v_aug (row 64 of the AV
    accumulator); reciprocal runs as [128,8] (DVE reciprocal costs
    ~6.5ns per free-column), reshaped via SBUF->SBUF DMA; broadcast
    across the 64 dim partitions via a DRAM round trip with
    partition-stride-0 read. The final block broadcasts on-chip (K=1
    matmul).
  * Stage B group B (q/k heads 2,3), v chunks 4-15 and the output
    projection are issued as closures interleaved one-per-unit into the
    attention loop to fill the ACT-bound PE gaps; ~6 closures are held
    back for the final normalize window so the PE (and its HAM clock)
    stays busy while the last block's reciprocal chain runs.
"""

import os
import sys
import types

for _p in ("/opt/trn_rl_repo", "/root/.axon_site", "/root/.axon_site/_ro/trn_rl_repo"):
    if os.path.isdir(_p) and _p not in sys.path:
        sys.path.append(_p)

import numpy as np
import ml_dtypes

import concourse.bacc as bacc
import concourse.bass as bass
import concourse.mybir as mybir
import concourse.tile as tile
from concourse.bass_utils import run_bass_kernel_spmd

# ── problem constants (hardcoded; spec.json not available at grade time) ──
B, T, C = 2, 2048, 1024
H, D = 16, 64
N_CORES = 8
HPG = 4                 # heads per group (per core)
CG = HPG * D            # 256 channels per head-group
NT = T // 512           # 4 query chunks of 512
KC = C // 128           # 8 contraction tiles for C
KP = KC // 2            # 4 fp8 DoubleRow K-pair chunks
VW = HPG * 65           # v tile width: 4x(64 dims + ones col) = 260
SC = 64.0               # fp8 weight pre-scale (power of two)

F32 = mybir.dt.float32
BF16 = mybir.dt.bfloat16
F8 = mybir.dt.float8e4
EXP = mybir.ActivationFunctionType.Exp
DR = mybir.MatmulPerfMode.DoubleRow
MULT = mybir.AluOpType.mult
ADD = mybir.AluOpType.add
NPBF = ml_dtypes.bfloat16
NPF8 = ml_dtypes.float8_e4m3fn

_trace_flag = [False]   # test.py can flip this to capture a profile
_last_results = [None]


def _ensure_ntff_hook():
    """Install the NTFF profile hook shim (container's antenv lacks it)."""
    if "antenv.axon_hooks" in sys.modules:
        return
    try:
        from trn_agent_boot.trn_boot import _ntff_profile_via_ctypes
    except Exception:
        return
    mod = types.ModuleType("antenv.axon_hooks")
    hook = [None]
    mod.set_axon_ntff_profile_hook = lambda h: hook.__setitem__(0, h)
    mod.get_axon_ntff_profile_hook = lambda: hook[0]
    sys.modules["antenv.axon_hooks"] = mod
    so = "/opt/axon/libaxon_pjrt.so"
    if os.path.exists(so):
        mod.set_axon_ntff_profile_hook(_ntff_profile_via_ctypes(so))


def build_nc():
    nc = bacc.Bacc("TRN2", target_bir_lowering=False, debug=False,
                   num_devices=N_CORES)

    # fp8 q/k weights, host-packed as [p, kp, s, f] -> [128, KP*2*2CG]
    wqk8_d = nc.dram_tensor("wqk8", [128, KP * 2 * 2 * CG], F8,
                            kind="ExternalInput").ap()
    xt_d = nc.dram_tensor("xt", [C, T], BF16, kind="ExternalInput").ap()
    wv_d = nc.dram_tensor("wv", [C, VW], BF16, kind="ExternalInput").ap()
    bqk_d = nc.dram_tensor("bqk", [2 * CG, 1], F32, kind="ExternalInput").ap()
    bvf_d = nc.dram_tensor("bvf", [128, VW], BF16, kind="ExternalInput").ap()
    wp_d = nc.dram_tensor("wp", [CG, C], BF16, kind="ExternalInput").ap()
    # tri2[k, s, m] = 1.0 where k <= m else 0 (causal diag strip mask)
    tri2_d = nc.dram_tensor("tri2", [128, 256], BF16, kind="ExternalInput").ap()
    ones16_d = nc.dram_tensor("ones16", [1, 64], BF16, kind="ExternalInput").ap()
    yt_d = nc.dram_tensor("yt", [C, T], BF16, kind="ExternalOutput").ap()
    rec_d = nc.dram_tensor("rec_scratch", [32, 512], F32).ap()

    with tile.TileContext(nc) as tc:
        with tc.tile_pool(name="const", bufs=1) as cp:
            # ── persistent SBUF residents ──
            wqk8 = cp.tile([128, KP, 2, 2 * CG], F8, tag="wqk8")
            xtp = [cp.tile([128, 2, T], BF16, tag=f"xtp{kp}", name=f"xtp{kp}")
                   for kp in range(KP)]
            xt8 = [cp.tile([128, 2, T], F8, tag=f"xt8{kp}", name=f"xt8{kp}")
                   for kp in range(KP)]
            wv = cp.tile([128, KC, VW], BF16, tag="wv")
            bqk = cp.tile([128, 4], F32, tag="bqk")
            bvf = cp.tile([128, VW], BF16, tag="bvf")
            wp = cp.tile([128, 2, C], BF16, tag="wp")
            tri2 = cp.tile([128, 2, 128], BF16, tag="tri2")
            ones16 = cp.tile([1, 64], BF16, tag="ones16")
            # qk[0]=q heads01, qk[1]=q heads23, qk[2]=k heads01, qk[3]=k heads23
            # (head pair laid out as rows 0-63 / 64-127 of the tile)
            qk = [cp.tile([128, T], BF16, tag=f"qk{m}", name=f"qk{m}")
                  for m in range(4)]
            v_sb = [cp.tile([128, VW], BF16, tag=f"v{m}", name=f"v{m}")
                    for m in range(T // 128)]
            outT = [cp.tile([128, T], BF16, tag=f"outT{k}", name=f"outT{k}")
                    for k in range(2)]

            # ── input DMA: few, large descriptors; x + q/k weights first ──
            nc.sync.dma_start(wqk8[:], wqk8_d[:, :])
            for kp in range(KP):
                nc.sync.dma_start(
                    xtp[kp][:],
                    bass.AP(xt_d.tensor, 256 * T * kp,
                            [[T, 128], [128 * T, 2], [1, T]]))
            nc.sync.dma_start(
                wv[:], bass.AP(wv_d.tensor, 0,
                               [[VW, 128], [128 * VW, KC], [1, VW]]))
            nc.sync.dma_start(
                bqk[:], bass.AP(bqk_d.tensor, 0, [[1, 128], [128, 4]]))
            nc.sync.dma_start(bvf[:], bvf_d[:])
            nc.sync.dma_start(tri2[:], tri2_d[:])
            nc.sync.dma_start(
                wp[:], bass.AP(wp_d.tensor, 0,
                               [[C, 128], [128 * C, 2], [1, C]]))
            nc.sync.dma_start(ones16[:], ones16_d[:])

            # on-chip bf16 -> fp8 casts for the q/k DoubleRow path
            for kp in range(KP):
                for s in range(2):
                    nc.vector.tensor_copy(xt8[kp][:, s, :], xtp[kp][:, s, :])

            # ── stage A: q,k for heads 0,1 (mf 0 and 2). kp-OUTER across 8
            # concurrent PSUM groups so the PE tracks the x DMA stream. ──
            with tc.tile_pool(name="psA", bufs=1, space="PSUM") as psA:
                pA = {}
                for mf in (0, 2):
                    for nt in range(NT):
                        pA[(mf, nt)] = psA.tile([128, 512], F32,
                                                tag=f"pA{mf}_{nt}",
                                                name=f"pA{mf}_{nt}")
                for kp in range(KP):
                    for nt in range(NT):
                        for mf in (0, 2):
                            nc.tensor.matmul(
                                pA[(mf, nt)][:],
                                wqk8[:, kp, :, 128 * mf:128 * (mf + 1)],
                                xt8[kp][:, :, 512 * nt:512 * (nt + 1)],
                                start=(kp == 0), stop=(kp == KP - 1),
                                perf_mode=DR)
                for mf in (0, 2):
                    for nt in range(NT):
                        nc.vector.tensor_scalar(
                            qk[mf][:, 512 * nt:512 * (nt + 1)],
                            pA[(mf, nt)][:], 1.0 / SC, bqk[:, mf:mf + 1],
                            op0=MULT, op1=ADD)

            # ── stage C part 1: v chunks 0-3 = xt.T @ wv; descale + bias via
            # one DVE scalar_tensor_tensor ──
            with tc.tile_pool(name="psC", bufs=4, space="PSUM") as psC:
                for mt in range(4):
                    ps = psC.tile([128, VW], F32, tag="psv", name=f"psv{mt}")
                    for k in range(KC):
                        nc.tensor.matmul(
                            ps[:], xtp[k // 2][:, k % 2,
                                               128 * mt:128 * (mt + 1)],
                            wv[:, k, :], start=(k == 0), stop=(k == KC - 1))
                    nc.vector.scalar_tensor_tensor(
                        v_sb[mt][:], ps[:], 1.0, bvf[:],
                        op0=MULT, op1=ADD)

            # ── stage D: attention, two head-pairs. j outer, key chunk i
            # inner. Foreign PE work (stage B group B = q,k heads 2,3;
            # stage C tails; stage E output projection) interleaved one
            # closure per unit. ──
            with (
                tc.tile_pool(name="ptp", bufs=2, space="PSUM") as ptp,
                tc.tile_pool(name="avpp", bufs=1, space="PSUM") as avpp,
                tc.tile_pool(name="fxp", bufs=2, space="PSUM") as fxp,
                tc.tile_pool(name="etp", bufs=4) as etp,
                tc.tile_pool(name="rawp", bufs=2) as rawp,
                tc.tile_pool(name="recp", bufs=2) as recp,
                tc.tile_pool(name="bcp", bufs=2) as bcp,
                tc.tile_pool(name="otp", bufs=4) as otp,
            ):
                foreign = []

                # stage C tail closures: v chunks 4-15, two closures of 4
                # K-steps each.
                def mk_c(mt):
                    holder = {}
                    cls = []
                    for half in range(2):
                        def ccl(mt=mt, half=half, holder=holder):
                            if half == 0:
                                holder["ps"] = fxp.tile(
                                    [128, 512], F32, tag="fx",
                                    name=f"pc{mt}")
                            ps = holder["ps"]
                            for k in range(4 * half, 4 * half + 4):
                                nc.tensor.matmul(
                                    ps[:, 0:VW],
                                    xtp[k // 2][:, k % 2,
                                                128 * mt:128 * (mt + 1)],
                                    wv[:, k, :], start=(k == 0),
                                    stop=(k == KC - 1))
                            if half == 1:
                                nc.vector.scalar_tensor_tensor(
                                    v_sb[mt][:], ps[:, 0:VW], 1.0, bvf[:],
                                    op0=MULT, op1=ADD)
                        cls.append(ccl)
                    return cls

                for mt in range(4, 12):
                    foreign.extend(mk_c(mt))

                # stage B group B closures: mf 1 (q23) and 3 (k23), two
                # closures of 2 DoubleRow K-pair steps each.
                for mf in (1, 3):
                    for nt in range(NT):
                        holder = {}
                        for half in range(2):
                            def bgc(mf=mf, nt=nt, half=half, holder=holder):
                                if half == 0:
                                    holder["ps"] = fxp.tile(
                                        [128, 512], F32, tag="fx",
                                        name=f"bg{mf}_{nt}")
                                ps = holder["ps"]
                                for kp in (2 * half, 2 * half + 1):
                                    nc.tensor.matmul(
                                        ps[:],
                                        wqk8[:, kp, :,
                                             128 * mf:128 * (mf + 1)],
                                        xt8[kp][:, :,
                                                512 * nt:512 * (nt + 1)],
                                        start=(kp == 0), stop=(kp == KP - 1),
                                        perf_mode=DR)
                                if half == 1:
                                    nc.vector.tensor_scalar(
                                        qk[mf][:, 512 * nt:512 * (nt + 1)],
                                        ps[:], 1.0 / SC, bqk[:, mf:mf + 1],
                                        op0=MULT, op1=ADD)
                            foreign.append(bgc)
                for mt in range(12, T // 128):
                    foreign.extend(mk_c(mt))

                def mk_e(mo, nt):
                    def ecl():
                        ps = fxp.tile([128, 512], F32, tag="fx",
                                      name=f"pe{mo}_{nt}")
                        for k in range(2):
                            nc.tensor.matmul(
                                ps[:], wp[:, k, 128 * mo:128 * (mo + 1)],
                                outT[k][:, 512 * nt:512 * (nt + 1)],
                                start=(k == 0), stop=(k == 1))
                        ot = otp.tile([128, 512], BF16, tag="ot",
                                      name=f"ot{mo}_{nt}")
                        if nt == 3 and mo % 2 == 0:
                            # tail chunk: ACT is idle by then; alternate
                            # with DVE so neither queue paces the drain
                            nc.scalar.copy(ot[:], ps[:])
                        else:
                            nc.vector.tensor_copy(ot[:], ps[:])
                        nc.sync.dma_start(
                            yt_d[128 * mo:128 * (mo + 1),
                                 512 * nt:512 * (nt + 1)], ot[:])
                    return ecl

                def do_av(p, j, i, c0, et, avp0, avp1):
                    last = (i == 4 * j + 3)
                    nc.tensor.matmul(
                        avp0[:, c0:512],
                        v_sb[i][:, 130 * p:130 * p + 65],
                        et[:, 0, c0:512], start=(i == 0), stop=last)
                    nc.tensor.matmul(
                        avp1[:, c0:512],
                        v_sb[i][:, 130 * p + 65:130 * p + 130],
                        et[:, 1, c0:512],
                        start=(i == 0), stop=last)

                def normalize(p, j, avp0, avp1):
                    # DVE reciprocal costs ~6.5ns/free-column no matter
                    # the partition count, so reshape the 2x512 denoms
                    # to [128,8] via SBUF->SBUF DMA (26ns recip). Raw
                    # copies come first so both avp banks free
                    # immediately for the next j's accumulation.
                    u2 = 8 * p + 2 * j
                    raws = []
                    den2 = recp.tile([128, 8], F32, tag="den2",
                                     name=f"den2_{p}_{j}")
                    for hh in range(2):
                        avp = avp0 if hh == 0 else avp1
                        raw = rawp.tile([65, 512], F32, tag=f"raw{hh}",
                                        name=f"raw{p}_{hh}_{j}")
                        nc.vector.tensor_copy(raw[:], avp[:])
                        nc.sync.dma_start(den2[64 * hh:64 * hh + 64, :],
                                          raw[64:65, :])
                        raws.append(raw)
                    rec2 = recp.tile([128, 8], F32, tag="rec2",
                                     name=f"rec2_{p}_{j}")
                    nc.vector.reciprocal(rec2[:], den2[:])
                    if p == 1 and j == 3:
                        # final block: keep the whole chain on-chip
                        # (sb->sb un-reshape + K=1 PE broadcast, all
                        # bf16 — fp32 matmuls lower to 4-pass) — a
                        # DRAM round trip would be fully exposed here.
                        rec2b = recp.tile([128, 8], BF16, tag="rec2b",
                                          name="rec2b")
                        nc.vector.tensor_copy(rec2b[:], rec2[:])
                        rec_row = recp.tile([1, 1024], BF16,
                                            tag="recrow", name="recrow")
                        nc.sync.dma_start(rec_row[:], rec2b[:])
                        for hh in range(2):
                            bcps = fxp.tile([128, 512], F32, tag="fx",
                                            name=f"bcps{hh}")
                            nc.tensor.matmul(
                                bcps[0:64, :], ones16[:],
                                rec_row[0:1, 512 * hh:512 * (hh + 1)],
                                start=True, stop=True)
                            nc.vector.tensor_mul(
                                outT[p][64 * hh:64 * hh + 64,
                                        512 * j:512 * (j + 1)],
                                raws[hh][0:64, :], bcps[0:64, :])
                    else:
                        nc.sync.dma_start(
                            bass.AP(rec_d.tensor, u2 * 512,
                                    [[8, 128], [1, 8]]), rec2[:])
                        for hh in range(2):
                            bc = bcp.tile([64, 512], F32, tag=f"bc{hh}",
                                          name=f"bc{p}_{hh}_{j}")
                            nc.sync.dma_start(
                                bc[:], bass.AP(rec_d.tensor,
                                               (u2 + hh) * 512,
                                               [[0, 64], [1, 512]]))
                            nc.vector.tensor_mul(
                                outT[p][64 * hh:64 * hh + 64,
                                        512 * j:512 * (j + 1)],
                                raws[hh][0:64, :], bc[:])
                    if p == 1:
                        for mo in range(8):
                            foreign.append(mk_e(mo, j))

                # flattened unit stream, software-pipelined ACROSS j (and
                # pair) boundaries: the next block's attT+exp issue before
                # the previous block's last AV flush and normalize, so the
                # ACT engine never waits at a block boundary.
                units = [(p, j, i) for p in range(2) for j in range(NT)
                         for i in range(4 * j + 4)]
                n_units = len(units)
                pending = None
                prev_norm = None
                avp0 = avp1 = None
                for uidx, (p, j, i) in enumerate(units):
                    qt = qk[p]
                    kt = qk[2 + p]
                    if i == 0:
                        avp0 = avpp.tile([65, 512], F32, tag="avp0",
                                         name=f"avp0_{p}_{j}")
                        avp1 = avpp.tile([65, 512], F32, tag="avp1",
                                         name=f"avp1_{p}_{j}")
                    diag = i >= 4 * j
                    c0 = 128 * (i - 4 * j) if diag else 0
                    pt = ptp.tile([128, 2, 512], F32, tag="pt",
                                  name=f"pt{p}_{j}_{i}")
                    # attT: both heads concurrently (row halves of the PE
                    # array) into the two 512-col blocks of one 2-bank
                    # PSUM tile; ONE exp ACTIVATE covers both via a 3D AP.
                    nc.tensor.matmul(
                        pt[:, 0, c0:512],
                        kt[0:64, 128 * i:128 * (i + 1)],
                        qt[0:64, 512 * j + c0:512 * (j + 1)],
                        start=True, stop=True)
                    nc.tensor.matmul(
                        pt[:, 1, c0:512],
                        kt[64:128, 128 * i:128 * (i + 1)],
                        qt[64:128, 512 * j + c0:512 * (j + 1)],
                        start=True, stop=True)
                    et = etp.tile([128, 2, 512], BF16, tag="et",
                                  name=f"et{p}_{j}_{i}")
                    nc.scalar.activation(et[:, :, c0:512],
                                         pt[:, :, c0:512], EXP)
                    if diag:
                        # causal mask: zero the upper triangle of the
                        # 128-wide diag strip (both heads in one DVE op,
                        # hidden in the exp->AV lag slack)
                        nc.vector.tensor_mul(et[:, :, c0:c0 + 128],
                                             et[:, :, c0:c0 + 128],
                                             tri2[:])
                    # foreign closure BEFORE the lagged AV: gives the
                    # previous unit's exp a few hundred ns to finish so the
                    # AV never stalls the PE queue on the ACT engine.
                    # Near the end, hold ~6 closures back so the PE stays
                    # busy (and HAM stays warm) through the final
                    # normalize chain.
                    if foreign and (uidx < n_units - 10 or len(foreign) > 6):
                        foreign.pop(0)()
                    if pending is not None:
                        do_av(*pending)
                    pending = (p, j, i, c0, et, avp0, avp1)
                    if prev_norm is not None:
                        normalize(*prev_norm)
                        prev_norm = None
                    if i == 4 * j + 3:
                        prev_norm = (p, j, avp0, avp1)
                do_av(*pending)
                normalize(*prev_norm)

                # drain any remaining foreign work (reserved closures
                # first, then the E chunks for j=3)
                while foreign:
                    foreign.pop(0)()

    nc.compile()
    return nc


def _pack8(a):
    """[C, F] f32 -> [128, KP*2*F] fp8 host layout [p, kp, s, f]."""
    Cd, F = a.shape
    return np.ascontiguousarray(
        a.reshape(KP, 2, 128, F).transpose(2, 0, 1, 3).reshape(128, -1)
    ).astype(NPF8)


def _shard_inputs(x, w_qkv, b_qkv, w_proj):
    scale = 1.0 / np.sqrt(D)   # 0.125, exact power of two
    in_maps = []
    k_idx = np.arange(128)[:, None]
    m_idx = np.arange(128)[None, :]
    tri = (k_idx <= m_idx).astype(np.float32)
    tri2 = np.concatenate([tri, tri], axis=1).astype(NPBF)
    for core in range(N_CORES):
        b, g = divmod(core, HPG)
        qs = slice(CG * g, CG * (g + 1))
        ks = slice(C + CG * g, C + CG * (g + 1))
        vs = slice(2 * C + CG * g, 2 * C + CG * (g + 1))
        wqk = np.concatenate([w_qkv[qs] * scale, w_qkv[ks]], axis=0).T
        bqk = np.concatenate([b_qkv[qs] * scale, b_qkv[ks]])[:, None]
        wv_base = w_qkv[vs].T          # [C, 256]
        wv = np.zeros((C, VW), np.float32)
        bv = np.zeros((1, VW), np.float32)
        for h in range(HPG):
            wv[:, 65 * h:65 * h + 64] = wv_base[:, 64 * h:64 * h + 64]
            bv[0, 65 * h:65 * h + 64] = b_qkv[vs][64 * h:64 * h + 64]
            bv[0, 65 * h + 64] = 1.0
        in_maps.append({
            "wqk8": _pack8(wqk * SC),
            "xt": np.ascontiguousarray(x[b].T).astype(NPBF),
            "wv": wv.astype(NPBF),
            "bqk": np.ascontiguousarray(bqk, np.float32),
            "bvf": np.broadcast_to(bv, (128, VW)).astype(NPBF),
            "wp": np.ascontiguousarray(
                w_proj[:, CG * g:CG * (g + 1)].T).astype(NPBF),
            "tri2": tri2,
            "ones16": np.ones((1, 64), NPBF),
        })
    return in_maps


def kernel(x, w_qkv, b_qkv, w_proj, b_proj):
    x = np.asarray(x, np.float32)
    w_qkv = np.asarray(w_qkv, np.float32)
    b_qkv = np.asarray(b_qkv, np.float32)
    w_proj = np.asarray(w_proj, np.float32)
    b_proj = np.asarray(b_proj, np.float32)

    nc = build_nc()
    in_maps = _shard_inputs(x, w_qkv, b_qkv, w_proj)
    if _trace_flag[0]:
        _ensure_ntff_hook()
    res = run_bass_kernel_spmd(nc, in_maps, core_ids=list(range(N_CORES)),
                               trace=_trace_flag[0])
    _last_results[0] = res

    y = np.empty((B, T, C), np.float32)
    for b in range(B):
        acc = np.zeros((C, T), np.float32)
        for g in range(HPG):
            acc += np.asarray(res.results[HPG * b + g]["yt"], np.float32)
        y[b] = acc.T + b_proj[None, :]
    return y


# revision 15
# speedup vs baseline: 1.0098x; 1.0098x over previous
"""Causal self-attention (B=2, T=2048, C=1024, H=16, D=64) on 8 trn2 cores.

Sharding: core c -> batch b = c // 4, head-group g = c % 4 (4 heads each).
Data-parallel over B, tensor-parallel (Megatron) over heads for the
qkv / proj linears. Each core computes its head-group's attention and a
partial output projection; the host sums the 4 partials per batch and
adds the proj bias.

Structure (v1, from the ~161us baseline):
  * Inputs batched into ~11 DMA descriptors (descriptor issue on the Sync
    engine costs ~600ns each; the old ~30 descriptors serialized the
    startup). x is loaded ONCE in bf16; the fp8 copy for the q/k path is
    cast on-chip by the DVE (saves 2.1MB of HBM traffic on the critical
    startup path).
  * q/k projection (fp8 DoubleRow, weights pre-scaled x64 on host) runs
    kp-OUTER across 8 concurrent PSUM accumulation groups so the PE
    tracks the x DMA stream as chunk-pairs land.
  * v projection: first 4 row-chunks serial (k-inner), rest moved into
    the attention foreign queue.
  * Attention as 2 head-PAIRS, pt/et tiles laid out [128, 2, 512]
    (h0/h1 blocks); ONE exp ACTIVATE covers both heads via a 3D AP.
  * Causal mask: DVE multiply of the exp output's 128-wide diag strip by
    a constant 0/1 triangular [128, 2, 128] mask - off the PE entirely
    (the old identity-matmul mask cost ~417ns of LDW+MM per diag unit),
    hidden in the exp->AV lag slack.
  * Softmax denominator from a ones-column in v_aug (row 64 of the AV
    accumulator); reciprocal runs as [128,8] (DVE reciprocal costs
    ~6.5ns per free-column), reshaped via SBUF->SBUF DMA; broadcast
    across the 64 dim partitions via a DRAM round trip with
    partition-stride-0 read. The final block broadcasts on-chip (K=1
    matmul).
  * Stage B group B (q/k heads 2,3), v chunks 4-15 and the output
    projection are issued as closures interleaved one-per-unit into the
    attention loop to fill the ACT-bound PE gaps; ~6 closures are held
    back for the final normalize window so the PE (and its HAM clock)
    stays busy while the last block's reciprocal chain runs.
"""

import os
import sys
import types

for _p in ("/opt/trn_rl_repo", "/root/.axon_site", "/root/.axon_site/_ro/trn_rl_repo"):
    if os.path.isdir(_p) and _p not in sys.path:
        sys.path.append(_p)

import numpy as np
import ml_dtypes

import concourse.bacc as bacc
import concourse.bass as bass
import concourse.mybir as mybir
import concourse.tile as tile
from concourse.bass_utils import run_bass_kernel_spmd

# ── problem constants (hardcoded; spec.json not available at grade time) ──
B, T, C = 2, 2048, 1024
H, D = 16, 64
N_CORES = 8
HPG = 4                 # heads per group (per core)
CG = HPG * D            # 256 channels per head-group
NT = T // 512           # 4 query chunks of 512
KC = C // 128           # 8 contraction tiles for C
KP = KC // 2            # 4 fp8 DoubleRow K-pair chunks
VW = HPG * 65           # v tile width: 4x(64 dims + ones col) = 260
SC = 64.0               # fp8 weight pre-scale (power of two)

F32 = mybir.dt.float32
BF16 = mybir.dt.bfloat16
F8 = mybir.dt.float8e4
EXP = mybir.ActivationFunctionType.Exp
DR = mybir.MatmulPerfMode.DoubleRow
MULT = mybir.AluOpType.mult
ADD = mybir.AluOpType.add
NPBF = ml_dtypes.bfloat16
NPF8 = ml_dtypes.float8_e4m3fn

_trace_flag = [False]   # test.py can flip this to capture a profile
_last_results = [None]


def _ensure_ntff_hook():
    """Install the NTFF profile hook shim (container's antenv lacks it)."""
    if "antenv.axon_hooks" in sys.modules:
        return
    try:
        from trn_agent_boot.trn_boot import _ntff_profile_via_ctypes
    except Exception:
        return
    mod = types.ModuleType("antenv.axon_hooks")
    hook = [None]
    mod.set_axon_ntff_profile_hook = lambda h: hook.__setitem__(0, h)
    mod.get_axon_ntff_profile_hook = lambda: hook[0]
    sys.modules["antenv.axon_hooks"] = mod
    so = "/opt/axon/libaxon_pjrt.so"
    if os.path.exists(so):
        mod.set_axon_ntff_profile_hook(_ntff_profile_via_ctypes(so))


def build_nc():
    nc = bacc.Bacc("TRN2", target_bir_lowering=False, debug=False,
                   num_devices=N_CORES)

    # fp8 q/k weights, host-packed as [p, kp, s, f] -> [128, KP*2*2CG]
    wqk8_d = nc.dram_tensor("wqk8", [128, KP * 2 * 2 * CG], F8,
                            kind="ExternalInput").ap()
    # fp8 seed for the kp=3 chunk of x: the first DoubleRow matmul does not
    # wait for a bf16 chunk + cast, and the LAST-arriving bf16 chunk (xtp3)
    # is not on stage A's critical path; kp 0-2 are cast from xtp on-chip.
    xt8s_d = nc.dram_tensor("xt8s", [128, 2 * T], F8,
                            kind="ExternalInput").ap()
    xt_d = nc.dram_tensor("xt", [C, T], BF16, kind="ExternalInput").ap()
    wv_d = nc.dram_tensor("wv", [C, VW], BF16, kind="ExternalInput").ap()
    bqk_d = nc.dram_tensor("bqk", [2 * CG, 1], F32, kind="ExternalInput").ap()
    bvf_d = nc.dram_tensor("bvf", [128, VW], BF16, kind="ExternalInput").ap()
    wp_d = nc.dram_tensor("wp", [CG, C], BF16, kind="ExternalInput").ap()
    # tri2[k, s, m] = 1.0 where k <= m else 0 (causal diag strip mask)
    tri2_d = nc.dram_tensor("tri2", [128, 256], BF16, kind="ExternalInput").ap()
    ones16_d = nc.dram_tensor("ones16", [1, 64], BF16, kind="ExternalInput").ap()
    yt_d = nc.dram_tensor("yt", [C, T], BF16, kind="ExternalOutput").ap()
    rec_d = nc.dram_tensor("rec_scratch", [32, 512], F32).ap()

    with tile.TileContext(nc) as tc:
        with tc.tile_pool(name="const", bufs=1) as cp:
            # ── persistent SBUF residents ──
            wqk8 = cp.tile([128, KP, 2, 2 * CG], F8, tag="wqk8")
            xtp = [cp.tile([128, 2, T], BF16, tag=f"xtp{kp}", name=f"xtp{kp}")
                   for kp in range(KP)]
            xt8 = [cp.tile([128, 2, T], F8, tag=f"xt8{kp}", name=f"xt8{kp}")
                   for kp in range(KP)]
            warm = cp.tile([128, 128], BF16, tag="warm")
            wv = cp.tile([128, KC, VW], BF16, tag="wv")
            bqk = cp.tile([128, 4], F32, tag="bqk")
            bvf = cp.tile([128, VW], BF16, tag="bvf")
            wp = cp.tile([128, 2, C], BF16, tag="wp")
            tri2 = cp.tile([128, 2, 128], BF16, tag="tri2")
            ones16 = cp.tile([1, 64], BF16, tag="ones16")
            # qk[0]=q heads01, qk[1]=q heads23, qk[2]=k heads01, qk[3]=k heads23
            # (head pair laid out as rows 0-63 / 64-127 of the tile)
            qk = [cp.tile([128, T], BF16, tag=f"qk{m}", name=f"qk{m}")
                  for m in range(4)]
            v_sb = [cp.tile([128, VW], BF16, tag=f"v{m}", name=f"v{m}")
                    for m in range(T // 128)]
            outT = [cp.tile([128, T], BF16, tag=f"outT{k}", name=f"outT{k}")
                    for k in range(2)]

            # ── input DMA: few, large descriptors; x + q/k weights first ──
            nc.sync.dma_start(wqk8[:], wqk8_d[:, :])
            nc.sync.dma_start(xt8[3][:], xt8s_d[:])
            for kp in range(2):
                nc.sync.dma_start(
                    xtp[kp][:],
                    bass.AP(xt_d.tensor, 256 * T * kp,
                            [[T, 128], [128 * T, 2], [1, T]]))
            nc.sync.dma_start(
                wv[:], bass.AP(wv_d.tensor, 0,
                               [[VW, 128], [128 * VW, KC], [1, VW]]))
            for kp in range(2, KP):
                nc.sync.dma_start(
                    xtp[kp][:],
                    bass.AP(xt_d.tensor, 256 * T * kp,
                            [[T, 128], [128 * T, 2], [1, T]]))
            nc.sync.dma_start(
                bqk[:], bass.AP(bqk_d.tensor, 0, [[1, 128], [128, 4]]))
            nc.sync.dma_start(bvf[:], bvf_d[:])
            nc.sync.dma_start(tri2[:], tri2_d[:])
            nc.sync.dma_start(
                wp[:], bass.AP(wp_d.tensor, 0,
                               [[C, 128], [128 * C, 2], [1, C]]))
            nc.sync.dma_start(ones16[:], ones16_d[:])

            # ── HAM warmup: ~3.5us of dummy matmuls with no DMA deps so the
            # PE clock gate opens before the first real (DMA-gated) matmul ──
            nc.vector.memset(warm[:], 0)
            with tc.tile_pool(name="psW", bufs=1, space="PSUM") as psW:
                pw = psW.tile([128, 512], F32, tag="pw")
                for w in range(34):
                    nc.tensor.matmul(pw[:, 0:128], warm[:], warm[:],
                                     start=True, stop=True)

            # on-chip bf16 -> fp8 casts for the q/k DoubleRow path (kp 0-2)
            for kp in range(KP - 1):
                for s in range(2):
                    nc.vector.tensor_copy(xt8[kp][:, s, :], xtp[kp][:, s, :])

            # ── stage A: q,k for heads 0,1 (mf 0 and 2). kp-OUTER across 8
            # concurrent PSUM groups so the PE tracks the x DMA stream. ──
            with tc.tile_pool(name="psA", bufs=1, space="PSUM") as psA:
                pA = {}
                for mf in (0, 2):
                    for nt in range(NT):
                        pA[(mf, nt)] = psA.tile([128, 512], F32,
                                                tag=f"pA{mf}_{nt}",
                                                name=f"pA{mf}_{nt}")
                for kpi, kp in enumerate((3, 0, 1, 2)):
                    for nt in range(NT):
                        for mf in (0, 2):
                            nc.tensor.matmul(
                                pA[(mf, nt)][:],
                                wqk8[:, kp, :, 128 * mf:128 * (mf + 1)],
                                xt8[kp][:, :, 512 * nt:512 * (nt + 1)],
                                start=(kpi == 0), stop=(kpi == KP - 1),
                                perf_mode=DR)
                for mf in (0, 2):
                    for nt in range(NT):
                        nc.vector.tensor_scalar(
                            qk[mf][:, 512 * nt:512 * (nt + 1)],
                            pA[(mf, nt)][:], 1.0 / SC, bqk[:, mf:mf + 1],
                            op0=MULT, op1=ADD)

            # ── stage C part 1: v chunks 0-3 = xt.T @ wv; descale + bias via
            # one DVE scalar_tensor_tensor ──
            with tc.tile_pool(name="psC", bufs=4, space="PSUM") as psC:
                for mt in range(4):
                    ps = psC.tile([128, VW], F32, tag="psv", name=f"psv{mt}")
                    for k in range(KC):
                        nc.tensor.matmul(
                            ps[:], xtp[k // 2][:, k % 2,
                                               128 * mt:128 * (mt + 1)],
                            wv[:, k, :], start=(k == 0), stop=(k == KC - 1))
                    nc.vector.scalar_tensor_tensor(
                        v_sb[mt][:], ps[:], 1.0, bvf[:],
                        op0=MULT, op1=ADD)

            # ── stage D: attention, two head-pairs. j outer, key chunk i
            # inner. Foreign PE work (stage B group B = q,k heads 2,3;
            # stage C tails; stage E output projection) interleaved one
            # closure per unit. ──
            with (
                tc.tile_pool(name="ptp", bufs=2, space="PSUM") as ptp,
                tc.tile_pool(name="avpp", bufs=1, space="PSUM") as avpp,
                tc.tile_pool(name="fxp", bufs=2, space="PSUM") as fxp,
                tc.tile_pool(name="etp", bufs=4) as etp,
                tc.tile_pool(name="rawp", bufs=2) as rawp,
                tc.tile_pool(name="recp", bufs=2) as recp,
                tc.tile_pool(name="bcp", bufs=2) as bcp,
                tc.tile_pool(name="otp", bufs=4) as otp,
            ):
                foreign = []

                # stage C tail closures: v chunks 4-15, two closures of 4
                # K-steps each.
                def mk_c(mt):
                    holder = {}
                    cls = []
                    for half in range(2):
                        def ccl(mt=mt, half=half, holder=holder):
                            if half == 0:
                                holder["ps"] = fxp.tile(
                                    [128, 512], F32, tag="fx",
                                    name=f"pc{mt}")
                            ps = holder["ps"]
                            for k in range(4 * half, 4 * half + 4):
                                nc.tensor.matmul(
                                    ps[:, 0:VW],
                                    xtp[k // 2][:, k % 2,
                                                128 * mt:128 * (mt + 1)],
                                    wv[:, k, :], start=(k == 0),
                                    stop=(k == KC - 1))
                            if half == 1:
                                nc.vector.scalar_tensor_tensor(
                                    v_sb[mt][:], ps[:, 0:VW], 1.0, bvf[:],
                                    op0=MULT, op1=ADD)
                        cls.append(ccl)
                    return cls

                for mt in range(4, 12):
                    foreign.extend(mk_c(mt))

                # stage B group B closures: mf 1 (q23) and 3 (k23), two
                # closures of 2 DoubleRow K-pair steps each.
                for mf in (1, 3):
                    for nt in range(NT):
                        holder = {}
                        for half in range(2):
                            def bgc(mf=mf, nt=nt, half=half, holder=holder):
                                if half == 0:
                                    holder["ps"] = fxp.tile(
                                        [128, 512], F32, tag="fx",
                                        name=f"bg{mf}_{nt}")
                                ps = holder["ps"]
                                for kp in (2 * half, 2 * half + 1):
                                    nc.tensor.matmul(
                                        ps[:],
                                        wqk8[:, kp, :,
                                             128 * mf:128 * (mf + 1)],
                                        xt8[kp][:, :,
                                                512 * nt:512 * (nt + 1)],
                                        start=(kp == 0), stop=(kp == KP - 1),
                                        perf_mode=DR)
                                if half == 1:
                                    nc.vector.tensor_scalar(
                                        qk[mf][:, 512 * nt:512 * (nt + 1)],
                                        ps[:], 1.0 / SC, bqk[:, mf:mf + 1],
                                        op0=MULT, op1=ADD)
                            foreign.append(bgc)
                for mt in range(12, T // 128):
                    foreign.extend(mk_c(mt))

                def mk_e(mo, nt):
                    def ecl():
                        ps = fxp.tile([128, 512], F32, tag="fx",
                                      name=f"pe{mo}_{nt}")
                        for k in range(2):
                            nc.tensor.matmul(
                                ps[:], wp[:, k, 128 * mo:128 * (mo + 1)],
                                outT[k][:, 512 * nt:512 * (nt + 1)],
                                start=(k == 0), stop=(k == 1))
                        ot = otp.tile([128, 512], BF16, tag="ot",
                                      name=f"ot{mo}_{nt}")
                        if nt == 3 and mo % 2 == 0:
                            # tail chunk: ACT is idle by then; alternate
                            # with DVE so neither queue paces the drain
                            nc.scalar.copy(ot[:], ps[:])
                        else:
                            nc.vector.tensor_copy(ot[:], ps[:])
                        nc.sync.dma_start(
                            yt_d[128 * mo:128 * (mo + 1),
                                 512 * nt:512 * (nt + 1)], ot[:])
                    return ecl

                def do_av(p, j, i, c0, et, avp0, avp1):
                    last = (i == 4 * j + 3)
                    nc.tensor.matmul(
                        avp0[:, c0:512],
                        v_sb[i][:, 130 * p:130 * p + 65],
                        et[:, 0, c0:512], start=(i == 0), stop=last)
                    nc.tensor.matmul(
                        avp1[:, c0:512],
                        v_sb[i][:, 130 * p + 65:130 * p + 130],
                        et[:, 1, c0:512],
                        start=(i == 0), stop=last)

                def normalize(p, j, avp0, avp1):
                    # DVE reciprocal costs ~6.5ns/free-column no matter
                    # the partition count, so reshape the 2x512 denoms
                    # to [128,8] via SBUF->SBUF DMA (26ns recip). Raw
                    # copies come first so both avp banks free
                    # immediately for the next j's accumulation.
                    u2 = 8 * p + 2 * j
                    raws = []
                    den2 = recp.tile([128, 8], F32, tag="den2",
                                     name=f"den2_{p}_{j}")
                    for hh in range(2):
                        avp = avp0 if hh == 0 else avp1
                        raw = rawp.tile([65, 512], F32, tag=f"raw{hh}",
                                        name=f"raw{p}_{hh}_{j}")
                        nc.vector.tensor_copy(raw[:], avp[:])
                        nc.sync.dma_start(den2[64 * hh:64 * hh + 64, :],
                                          raw[64:65, :])
                        raws.append(raw)
                    rec2 = recp.tile([128, 8], F32, tag="rec2",
                                     name=f"rec2_{p}_{j}")
                    nc.vector.reciprocal(rec2[:], den2[:])
                    if p == 1 and j == 3:
                        # final block: keep the whole chain on-chip
                        # (sb->sb un-reshape + K=1 PE broadcast, all
                        # bf16 — fp32 matmuls lower to 4-pass) — a
                        # DRAM round trip would be fully exposed here.
                        rec2b = recp.tile([128, 8], BF16, tag="rec2b",
                                          name="rec2b")
                        nc.vector.tensor_copy(rec2b[:], rec2[:])
                        rec_row = recp.tile([1, 1024], BF16,
                                            tag="recrow", name="recrow")
                        nc.sync.dma_start(rec_row[:], rec2b[:])
                        for hh in range(2):
                            bcps = fxp.tile([128, 512], F32, tag="fx",
                                            name=f"bcps{hh}")
                            nc.tensor.matmul(
                                bcps[0:64, :], ones16[:],
                                rec_row[0:1, 512 * hh:512 * (hh + 1)],
                                start=True, stop=True)
                            nc.vector.tensor_mul(
                                outT[p][64 * hh:64 * hh + 64,
                                        512 * j:512 * (j + 1)],
                                raws[hh][0:64, :], bcps[0:64, :])
                    else:
                        nc.sync.dma_start(
                            bass.AP(rec_d.tensor, u2 * 512,
                                    [[8, 128], [1, 8]]), rec2[:])
                        for hh in range(2):
                            bc = bcp.tile([64, 512], F32, tag=f"bc{hh}",
                                          name=f"bc{p}_{hh}_{j}")
                            nc.sync.dma_start(
                                bc[:], bass.AP(rec_d.tensor,
                                               (u2 + hh) * 512,
                                               [[0, 64], [1, 512]]))
                            nc.vector.tensor_mul(
                                outT[p][64 * hh:64 * hh + 64,
                                        512 * j:512 * (j + 1)],
                                raws[hh][0:64, :], bc[:])
                    if p == 1:
                        for mo in range(8):
                            foreign.append(mk_e(mo, j))

                # flattened unit stream, software-pipelined ACROSS j (and
                # pair) boundaries: the next block's attT+exp issue before
                # the previous block's last AV flush and normalize, so the
                # ACT engine never waits at a block boundary.
                units = [(p, j, i) for p in range(2) for j in range(NT)
                         for i in range(4 * j + 4)]
                n_units = len(units)
                pending = None
                prev_norm = None
                avp0 = avp1 = None
                for uidx, (p, j, i) in enumerate(units):
                    qt = qk[p]
                    kt = qk[2 + p]
                    if i == 0:
                        avp0 = avpp.tile([65, 512], F32, tag="avp0",
                                         name=f"avp0_{p}_{j}")
                        avp1 = avpp.tile([65, 512], F32, tag="avp1",
                                         name=f"avp1_{p}_{j}")
                    diag = i >= 4 * j
                    c0 = 128 * (i - 4 * j) if diag else 0
                    pt = ptp.tile([128, 2, 512], F32, tag="pt",
                                  name=f"pt{p}_{j}_{i}")
                    # attT: both heads concurrently (row halves of the PE
                    # array) into the two 512-col blocks of one 2-bank
                    # PSUM tile; ONE exp ACTIVATE covers both via a 3D AP.
                    nc.tensor.matmul(
                        pt[:, 0, c0:512],
                        kt[0:64, 128 * i:128 * (i + 1)],
                        qt[0:64, 512 * j + c0:512 * (j + 1)],
                        start=True, stop=True)
                    nc.tensor.matmul(
                        pt[:, 1, c0:512],
                        kt[64:128, 128 * i:128 * (i + 1)],
                        qt[64:128, 512 * j + c0:512 * (j + 1)],
                        start=True, stop=True)
                    et = etp.tile([128, 2, 512], BF16, tag="et",
                                  name=f"et{p}_{j}_{i}")
                    nc.scalar.activation(et[:, :, c0:512],
                                         pt[:, :, c0:512], EXP)
                    # foreign closure BEFORE the lagged AV: gives the
                    # previous unit's exp a few hundred ns to finish so the
                    # AV never stalls the PE queue on the ACT engine.
                    # In the second half, hold ~8 closures back so the PE
                    # stays busy (and HAM stays warm) through the final
                    # normalize chain.
                    if foreign and (uidx < 44 or len(foreign) > 8):
                        foreign.pop(0)()
                    if pending is not None:
                        do_av(*pending)
                    pending = (p, j, i, c0, et, avp0, avp1)
                    if prev_norm is not None:
                        normalize(*prev_norm)
                        prev_norm = None
                    if i == 4 * j + 3:
                        prev_norm = (p, j, avp0, avp1)
                    if diag:
                        # causal mask: zero the upper triangle of the
                        # 128-wide diag strip (both heads in one DVE op,
                        # hidden in the exp->AV lag slack). Issued LAST in
                        # the unit so it does not block the strict-FIFO
                        # DVE queue while its exp is still running.
                        nc.vector.tensor_mul(et[:, :, c0:c0 + 128],
                                             et[:, :, c0:c0 + 128],
                                             tri2[:])
                do_av(*pending)
                normalize(*prev_norm)

                # drain any remaining foreign work (reserved closures
                # first, then the E chunks for j=3)
                while foreign:
                    foreign.pop(0)()

    nc.compile()
    return nc


def _pack8(a):
    """[C, F] f32 -> [128, KP*2*F] fp8 host layout [p, kp, s, f]."""
    Cd, F = a.shape
    return np.ascontiguousarray(
        a.reshape(KP, 2, 128, F).transpose(2, 0, 1, 3).reshape(128, -1)
    ).astype(NPF8)


def _shard_inputs(x, w_qkv, b_qkv, w_proj):
    scale = 1.0 / np.sqrt(D)   # 0.125, exact power of two
    in_maps = []
    k_idx = np.arange(128)[:, None]
    m_idx = np.arange(128)[None, :]
    tri = (k_idx <= m_idx).astype(np.float32)
    tri2 = np.concatenate([tri, tri], axis=1).astype(NPBF)
    for core in range(N_CORES):
        b, g = divmod(core, HPG)
        qs = slice(CG * g, CG * (g + 1))
        ks = slice(C + CG * g, C + CG * (g + 1))
        vs = slice(2 * C + CG * g, 2 * C + CG * (g + 1))
        wqk = np.concatenate([w_qkv[qs] * scale, w_qkv[ks]], axis=0).T
        bqk = np.concatenate([b_qkv[qs] * scale, b_qkv[ks]])[:, None]
        wv_base = w_qkv[vs].T          # [C, 256]
        wv = np.zeros((C, VW), np.float32)
        bv = np.zeros((1, VW), np.float32)
        for h in range(HPG):
            wv[:, 65 * h:65 * h + 64] = wv_base[:, 64 * h:64 * h + 64]
            bv[0, 65 * h:65 * h + 64] = b_qkv[vs][64 * h:64 * h + 64]
            bv[0, 65 * h + 64] = 1.0
        xb = np.ascontiguousarray(x[b].T)          # [C, T]
        xt8s = np.ascontiguousarray(
            xb[768:1024].reshape(2, 128, T).transpose(1, 0, 2)
            .reshape(128, 2 * T)).astype(NPF8)
        in_maps.append({
            "wqk8": _pack8(wqk * SC),
            "xt8s": xt8s,
            "xt": xb.astype(NPBF),
            "wv": wv.astype(NPBF),
            "bqk": np.ascontiguousarray(bqk, np.float32),
            "bvf": np.broadcast_to(bv, (128, VW)).astype(NPBF),
            "wp": np.ascontiguousarray(
                w_proj[:, CG * g:CG * (g + 1)].T).astype(NPBF),
            "tri2": tri2,
            "ones16": np.ones((1, 64), NPBF),
        })
    return in_maps


def kernel(x, w_qkv, b_qkv, w_proj, b_proj):
    x = np.asarray(x, np.float32)
    w_qkv = np.asarray(w_qkv, np.float32)
    b_qkv = np.asarray(b_qkv, np.float32)
    w_proj = np.asarray(w_proj, np.float32)
    b_proj = np.asarray(b_proj, np.float32)

    nc = build_nc()
    in_maps = _shard_inputs(x, w_qkv, b_qkv, w_proj)
    if _trace_flag[0]:
        _ensure_ntff_hook()
    res = run_bass_kernel_spmd(nc, in_maps, core_ids=list(range(N_CORES)),
                               trace=_trace_flag[0])
    _last_results[0] = res

    y = np.empty((B, T, C), np.float32)
    for b in range(B):
        acc = np.zeros((C, T), np.float32)
        for g in range(HPG):
            acc += np.asarray(res.results[HPG * b + g]["yt"], np.float32)
        y[b] = acc.T + b_proj[None, :]
    return y


# revision 19
# speedup vs baseline: 1.0313x; 1.0213x over previous
"""Causal self-attention (B=2, T=2048, C=1024, H=16, D=64) on 8 trn2 cores.

Sharding: core c -> batch b = c // 4, head-group g = c % 4 (4 heads each).
Data-parallel over B, tensor-parallel (Megatron) over heads for the
qkv / proj linears. Each core computes its head-group's attention and a
partial output projection; the host sums the 4 partials per batch and
adds the proj bias.

Structure (v1, from the ~161us baseline):
  * Inputs batched into ~11 DMA descriptors (descriptor issue on the Sync
    engine costs ~600ns each; the old ~30 descriptors serialized the
    startup). x is loaded ONCE in bf16; the fp8 copy for the q/k path is
    cast on-chip by the DVE (saves 2.1MB of HBM traffic on the critical
    startup path).
  * q/k projection (fp8 DoubleRow, weights pre-scaled x64 on host) runs
    kp-OUTER across 8 concurrent PSUM accumulation groups so the PE
    tracks the x DMA stream as chunk-pairs land.
  * v projection: first 4 row-chunks serial (k-inner), rest moved into
    the attention foreign queue.
  * Attention as 2 head-PAIRS, pt/et tiles laid out [128, 2, 512]
    (h0/h1 blocks); ONE exp ACTIVATE covers both heads via a 3D AP.
  * Causal mask: DVE multiply of the exp output's 128-wide diag strip by
    a constant 0/1 triangular [128, 2, 128] mask - off the PE entirely
    (the old identity-matmul mask cost ~417ns of LDW+MM per diag unit),
    hidden in the exp->AV lag slack.
  * Softmax denominator from a ones-column in v_aug (row 64 of the AV
    accumulator); reciprocal runs as [128,8] (DVE reciprocal costs
    ~6.5ns per free-column), reshaped via SBUF->SBUF DMA; broadcast
    across the 64 dim partitions via a DRAM round trip with
    partition-stride-0 read. The final block broadcasts on-chip (K=1
    matmul).
  * Stage B group B (q/k heads 2,3), v chunks 4-15 and the output
    projection are issued as closures interleaved one-per-unit into the
    attention loop to fill the ACT-bound PE gaps; ~6 closures are held
    back for the final normalize window so the PE (and its HAM clock)
    stays busy while the last block's reciprocal chain runs.
"""

import os
import sys
import types

for _p in ("/opt/trn_rl_repo", "/root/.axon_site", "/root/.axon_site/_ro/trn_rl_repo"):
    if os.path.isdir(_p) and _p not in sys.path:
        sys.path.append(_p)

import numpy as np
import ml_dtypes

import concourse.bacc as bacc
import concourse.bass as bass
import concourse.mybir as mybir
import concourse.tile as tile
from concourse.bass_utils import run_bass_kernel_spmd

# ── problem constants (hardcoded; spec.json not available at grade time) ──
B, T, C = 2, 2048, 1024
H, D = 16, 64
N_CORES = 8
HPG = 4                 # heads per group (per core)
CG = HPG * D            # 256 channels per head-group
NT = T // 512           # 4 query chunks of 512
KC = C // 128           # 8 contraction tiles for C
KP = KC // 2            # 4 fp8 DoubleRow K-pair chunks
VW = HPG * 65           # v tile width: 4x(64 dims + ones col) = 260
SC = 64.0               # fp8 weight pre-scale (power of two)

F32 = mybir.dt.float32
BF16 = mybir.dt.bfloat16
F8 = mybir.dt.float8e4
EXP = mybir.ActivationFunctionType.Exp
DR = mybir.MatmulPerfMode.DoubleRow
MULT = mybir.AluOpType.mult
ADD = mybir.AluOpType.add
NPBF = ml_dtypes.bfloat16
NPF8 = ml_dtypes.float8_e4m3fn

_trace_flag = [False]   # test.py can flip this to capture a profile
_last_results = [None]


def _ensure_ntff_hook():
    """Install the NTFF profile hook shim (container's antenv lacks it)."""
    if "antenv.axon_hooks" in sys.modules:
        return
    try:
        from trn_agent_boot.trn_boot import _ntff_profile_via_ctypes
    except Exception:
        return
    mod = types.ModuleType("antenv.axon_hooks")
    hook = [None]
    mod.set_axon_ntff_profile_hook = lambda h: hook.__setitem__(0, h)
    mod.get_axon_ntff_profile_hook = lambda: hook[0]
    sys.modules["antenv.axon_hooks"] = mod
    so = "/opt/axon/libaxon_pjrt.so"
    if os.path.exists(so):
        mod.set_axon_ntff_profile_hook(_ntff_profile_via_ctypes(so))


def build_nc():
    nc = bacc.Bacc("TRN2", target_bir_lowering=False, debug=False,
                   num_devices=N_CORES)

    # fp8 q/k weights, host-packed as [p, kp, s, f] -> [128, KP*2*2CG]
    wqk8_d = nc.dram_tensor("wqk8", [128, KP * 2 * 2 * CG], F8,
                            kind="ExternalInput").ap()
    # fp8 seed for the kp=3 chunk of x: the first DoubleRow matmul does not
    # wait for a bf16 chunk + cast, and the LAST-arriving bf16 chunk (xtp3)
    # is not on stage A's critical path; kp 0-2 are cast from xtp on-chip.
    xt8s_d = nc.dram_tensor("xt8s", [128, 2 * T], F8,
                            kind="ExternalInput").ap()
    xt_d = nc.dram_tensor("xt", [C, T], BF16, kind="ExternalInput").ap()
    wv_d = nc.dram_tensor("wv", [C, VW], BF16, kind="ExternalInput").ap()
    bqk_d = nc.dram_tensor("bqk", [2 * CG, 1], F32, kind="ExternalInput").ap()
    bvf_d = nc.dram_tensor("bvf", [128, VW], BF16, kind="ExternalInput").ap()
    wp_d = nc.dram_tensor("wp", [CG, C], BF16, kind="ExternalInput").ap()
    # tri2[k, s, m] = 1.0 where k <= m else 0 (causal diag strip mask)
    tri2_d = nc.dram_tensor("tri2", [128, 256], BF16, kind="ExternalInput").ap()
    ones16_d = nc.dram_tensor("ones16", [1, 64], BF16, kind="ExternalInput").ap()
    yt_d = nc.dram_tensor("yt", [C, T], BF16, kind="ExternalOutput").ap()
    rec_d = nc.dram_tensor("rec_scratch", [32, 512], F32).ap()

    with tile.TileContext(nc) as tc:
        with tc.tile_pool(name="const", bufs=1) as cp:
            # ── persistent SBUF residents ──
            wqk8 = cp.tile([128, KP, 2, 2 * CG], F8, tag="wqk8")
            xtp = [cp.tile([128, 2, T], BF16, tag=f"xtp{kp}", name=f"xtp{kp}")
                   for kp in range(KP)]
            xt8 = [cp.tile([128, 2, T], F8, tag=f"xt8{kp}", name=f"xt8{kp}")
                   for kp in range(KP)]
            warm = cp.tile([128, 128], BF16, tag="warm")
            wv = cp.tile([128, KC, VW], BF16, tag="wv")
            bqk = cp.tile([128, 4], F32, tag="bqk")
            bvf = cp.tile([128, VW], BF16, tag="bvf")
            wp = cp.tile([128, 2, C], BF16, tag="wp")
            tri2 = cp.tile([128, 2, 128], BF16, tag="tri2")
            ones16 = cp.tile([1, 64], BF16, tag="ones16")
            # qk[0]=q heads01, qk[1]=q heads23, qk[2]=k heads01, qk[3]=k heads23
            # (head pair laid out as rows 0-63 / 64-127 of the tile)
            qk = [cp.tile([128, T], BF16, tag=f"qk{m}", name=f"qk{m}")
                  for m in range(4)]
            v_sb = [cp.tile([128, VW], BF16, tag=f"v{m}", name=f"v{m}")
                    for m in range(T // 128)]
            outT = [cp.tile([128, T], BF16, tag=f"outT{k}", name=f"outT{k}")
                    for k in range(2)]

            # ── input DMA: few, large descriptors; x + q/k weights first ──
            nc.sync.dma_start(wqk8[:], wqk8_d[:, :])
            nc.sync.dma_start(xt8[3][:], xt8s_d[:])
            for kp in range(2):
                nc.sync.dma_start(
                    xtp[kp][:],
                    bass.AP(xt_d.tensor, 256 * T * kp,
                            [[T, 128], [128 * T, 2], [1, T]]))
            nc.sync.dma_start(
                wv[:], bass.AP(wv_d.tensor, 0,
                               [[VW, 128], [128 * VW, KC], [1, VW]]))
            for kp in range(2, KP):
                nc.sync.dma_start(
                    xtp[kp][:],
                    bass.AP(xt_d.tensor, 256 * T * kp,
                            [[T, 128], [128 * T, 2], [1, T]]))
            nc.sync.dma_start(
                bqk[:], bass.AP(bqk_d.tensor, 0, [[1, 128], [128, 4]]))
            nc.sync.dma_start(bvf[:], bvf_d[:])
            nc.sync.dma_start(tri2[:], tri2_d[:])
            nc.sync.dma_start(
                wp[:], bass.AP(wp_d.tensor, 0,
                               [[C, 128], [128 * C, 2], [1, C]]))
            nc.sync.dma_start(ones16[:], ones16_d[:])

            # ── HAM warmup: ~3.5us of dummy matmuls with no DMA deps so the
            # PE clock gate opens before the first real (DMA-gated) matmul ──
            nc.vector.memset(warm[:], 0)
            with tc.tile_pool(name="psW", bufs=1, space="PSUM") as psW:
                pw = psW.tile([128, 512], F32, tag="pw")
                for w in range(34):
                    nc.tensor.matmul(pw[:, 0:128], warm[:], warm[:],
                                     start=True, stop=True)

            # on-chip bf16 -> fp8 casts for the q/k DoubleRow path (kp 0-2),
            # split per T-half so stage A's nt 0-1 matmuls start one half-
            # cast earlier
            for kp in range(KP - 1):
                for s in range(2):
                    for h in range(2):
                        nc.vector.tensor_copy(
                            xt8[kp][:, s, 1024 * h:1024 * (h + 1)],
                            xtp[kp][:, s, 1024 * h:1024 * (h + 1)])

            # ── stage A: q,k for heads 0,1 (mf 0 and 2). kp-OUTER across 8
            # concurrent PSUM groups so the PE tracks the x DMA stream; the
            # fp8 seed chunk (kp=3) runs first with no cast dependency. ──
            with tc.tile_pool(name="psAC", bufs=1, space="PSUM") as psA:
                pA = {}
                for mf in (0, 2):
                    for nt in range(NT):
                        pA[(mf, nt)] = psA.tile([128, 512], F32,
                                                tag=f"pA{mf}_{nt}",
                                                name=f"pA{mf}_{nt}")
                for kpi, kp in enumerate((3, 0, 1, 2)):
                    for nt in range(NT):
                        for mf in (0, 2):
                            nc.tensor.matmul(
                                pA[(mf, nt)][:],
                                wqk8[:, kp, :, 128 * mf:128 * (mf + 1)],
                                xt8[kp][:, :, 512 * nt:512 * (nt + 1)],
                                start=(kpi == 0), stop=(kpi == KP - 1),
                                perf_mode=DR)

                # ── bias-adds + stage C part 1 (v chunks 0-3), interleaved
                # so attention's j=0 gates (qk nt=0, v_sb[0..3]) clear as
                # early as possible. psv(mt) reuses pA(0,mt)'s PSUM bank,
                # whose bias-add runs just before it. ──
                def bias(mf, nt):
                    nc.vector.tensor_scalar(
                        qk[mf][:, 512 * nt:512 * (nt + 1)],
                        pA[(mf, nt)][:], 1.0 / SC, bqk[:, mf:mf + 1],
                        op0=MULT, op1=ADD)

                bias(0, 0)
                bias(2, 0)
                for mt in range(4):
                    ps = psA.tile([128, 512], F32, tag=f"pA0_{mt}",
                                  name=f"psv{mt}")
                    for k in range(KC):
                        nc.tensor.matmul(
                            ps[:, 0:VW],
                            xtp[k // 2][:, k % 2, 128 * mt:128 * (mt + 1)],
                            wv[:, k, :], start=(k == 0), stop=(k == KC - 1))
                    nc.vector.scalar_tensor_tensor(
                        v_sb[mt][:], ps[:, 0:VW], 1.0, bvf[:],
                        op0=MULT, op1=ADD)
                    if mt < 3:
                        bias(0, mt + 1)
                for nt in range(1, NT):
                    bias(2, nt)

            # ── stage D: attention, two head-pairs. j outer, key chunk i
            # inner. Foreign PE work (stage B group B = q,k heads 2,3;
            # stage C tails; stage E output projection) interleaved one
            # closure per unit. ──
            with (
                tc.tile_pool(name="ptp", bufs=2, space="PSUM") as ptp,
                tc.tile_pool(name="avpp", bufs=1, space="PSUM") as avpp,
                tc.tile_pool(name="fxp", bufs=2, space="PSUM") as fxp,
                tc.tile_pool(name="etp", bufs=4) as etp,
                tc.tile_pool(name="rawp", bufs=2) as rawp,
                tc.tile_pool(name="recp", bufs=2) as recp,
                tc.tile_pool(name="bcp", bufs=2) as bcp,
                tc.tile_pool(name="otp", bufs=4) as otp,
            ):
                foreign = []

                # stage C tail closures: v chunks 4-15, two closures of 4
                # K-steps each.
                def mk_c(mt):
                    holder = {}
                    cls = []
                    for half in range(2):
                        def ccl(mt=mt, half=half, holder=holder):
                            if half == 0:
                                holder["ps"] = fxp.tile(
                                    [128, 512], F32, tag="fx",
                                    name=f"pc{mt}")
                            ps = holder["ps"]
                            for k in range(4 * half, 4 * half + 4):
                                nc.tensor.matmul(
                                    ps[:, 0:VW],
                                    xtp[k // 2][:, k % 2,
                                                128 * mt:128 * (mt + 1)],
                                    wv[:, k, :], start=(k == 0),
                                    stop=(k == KC - 1))
                            if half == 1:
                                nc.vector.scalar_tensor_tensor(
                                    v_sb[mt][:], ps[:, 0:VW], 1.0, bvf[:],
                                    op0=MULT, op1=ADD)
                        cls.append(ccl)
                    return cls

                for mt in range(4, 12):
                    foreign.extend(mk_c(mt))

                # stage B group B closures: mf 1 (q23) and 3 (k23), two
                # closures of 2 DoubleRow K-pair steps each.
                for mf in (1, 3):
                    for nt in range(NT):
                        holder = {}
                        for half in range(2):
                            def bgc(mf=mf, nt=nt, half=half, holder=holder):
                                if half == 0:
                                    holder["ps"] = fxp.tile(
                                        [128, 512], F32, tag="fx",
                                        name=f"bg{mf}_{nt}")
                                ps = holder["ps"]
                                for kp in (2 * half, 2 * half + 1):
                                    nc.tensor.matmul(
                                        ps[:],
                                        wqk8[:, kp, :,
                                             128 * mf:128 * (mf + 1)],
                                        xt8[kp][:, :,
                                                512 * nt:512 * (nt + 1)],
                                        start=(kp == 0), stop=(kp == KP - 1),
                                        perf_mode=DR)
                                if half == 1:
                                    nc.vector.tensor_scalar(
                                        qk[mf][:, 512 * nt:512 * (nt + 1)],
                                        ps[:], 1.0 / SC, bqk[:, mf:mf + 1],
                                        op0=MULT, op1=ADD)
                            foreign.append(bgc)
                for mt in range(12, T // 128):
                    foreign.extend(mk_c(mt))

                def mk_e(mo, nt):
                    def ecl():
                        ps = fxp.tile([128, 512], F32, tag="fx",
                                      name=f"pe{mo}_{nt}")
                        for k in range(2):
                            nc.tensor.matmul(
                                ps[:], wp[:, k, 128 * mo:128 * (mo + 1)],
                                outT[k][:, 512 * nt:512 * (nt + 1)],
                                start=(k == 0), stop=(k == 1))
                        ot = otp.tile([128, 512], BF16, tag="ot",
                                      name=f"ot{mo}_{nt}")
                        if nt == 3 and mo % 2 == 0:
                            # tail chunk: ACT is idle by then; alternate
                            # with DVE so neither queue paces the drain
                            nc.scalar.copy(ot[:], ps[:])
                        else:
                            nc.vector.tensor_copy(ot[:], ps[:])
                        nc.sync.dma_start(
                            yt_d[128 * mo:128 * (mo + 1),
                                 512 * nt:512 * (nt + 1)], ot[:])
                    return ecl

                def do_av(p, j, i, c0, et, avp0, avp1):
                    last = (i == 4 * j + 3)
                    nc.tensor.matmul(
                        avp0[:, c0:512],
                        v_sb[i][:, 130 * p:130 * p + 65],
                        et[:, 0, c0:512], start=(i == 0), stop=last)
                    nc.tensor.matmul(
                        avp1[:, c0:512],
                        v_sb[i][:, 130 * p + 65:130 * p + 130],
                        et[:, 1, c0:512],
                        start=(i == 0), stop=last)

                def normalize(p, j, avp0, avp1):
                    # DVE reciprocal costs ~6.5ns/free-column no matter
                    # the partition count, so reshape the 2x512 denoms
                    # to [128,8] via SBUF->SBUF DMA (26ns recip). Raw
                    # copies come first so both avp banks free
                    # immediately for the next j's accumulation.
                    u2 = 8 * p + 2 * j
                    raws = []
                    den2 = recp.tile([128, 8], F32, tag="den2",
                                     name=f"den2_{p}_{j}")
                    for hh in range(2):
                        avp = avp0 if hh == 0 else avp1
                        raw = rawp.tile([65, 512], F32, tag=f"raw{hh}",
                                        name=f"raw{p}_{hh}_{j}")
                        nc.vector.tensor_copy(raw[:], avp[:])
                        nc.sync.dma_start(den2[64 * hh:64 * hh + 64, :],
                                          raw[64:65, :])
                        raws.append(raw)
                    rec2 = recp.tile([128, 8], F32, tag="rec2",
                                     name=f"rec2_{p}_{j}")
                    nc.vector.reciprocal(rec2[:], den2[:])
                    if p == 1 and j == 3:
                        # final block: keep the whole chain on-chip
                        # (sb->sb un-reshape + K=1 PE broadcast, all
                        # bf16 — fp32 matmuls lower to 4-pass) — a
                        # DRAM round trip would be fully exposed here.
                        rec2b = recp.tile([128, 8], BF16, tag="rec2b",
                                          name="rec2b")
                        nc.vector.tensor_copy(rec2b[:], rec2[:])
                        rec_row = recp.tile([1, 1024], BF16,
                                            tag="recrow", name="recrow")
                        nc.sync.dma_start(rec_row[:], rec2b[:])
                        for hh in range(2):
                            bcps = fxp.tile([128, 512], F32, tag="fx",
                                            name=f"bcps{hh}")
                            nc.tensor.matmul(
                                bcps[0:64, :], ones16[:],
                                rec_row[0:1, 512 * hh:512 * (hh + 1)],
                                start=True, stop=True)
                            nc.vector.tensor_mul(
                                outT[p][64 * hh:64 * hh + 64,
                                        512 * j:512 * (j + 1)],
                                raws[hh][0:64, :], bcps[0:64, :])
                    else:
                        nc.sync.dma_start(
                            bass.AP(rec_d.tensor, u2 * 512,
                                    [[8, 128], [1, 8]]), rec2[:])
                        for hh in range(2):
                            bc = bcp.tile([64, 512], F32, tag=f"bc{hh}",
                                          name=f"bc{p}_{hh}_{j}")
                            nc.sync.dma_start(
                                bc[:], bass.AP(rec_d.tensor,
                                               (u2 + hh) * 512,
                                               [[0, 64], [1, 512]]))
                            nc.vector.tensor_mul(
                                outT[p][64 * hh:64 * hh + 64,
                                        512 * j:512 * (j + 1)],
                                raws[hh][0:64, :], bc[:])
                    if p == 1:
                        for mo in range(8):
                            foreign.append(mk_e(mo, j))

                # flattened unit stream, software-pipelined ACROSS j (and
                # pair) boundaries: the next block's attT+exp issue before
                # the previous block's last AV flush and normalize, so the
                # ACT engine never waits at a block boundary.
                units = [(p, j, i) for p in range(2) for j in range(NT)
                         for i in range(4 * j + 4)]
                n_units = len(units)
                pending = None
                prev_norm = None
                avp0 = avp1 = None
                for uidx, (p, j, i) in enumerate(units):
                    qt = qk[p]
                    kt = qk[2 + p]
                    if i == 0:
                        avp0 = avpp.tile([65, 512], F32, tag="avp0",
                                         name=f"avp0_{p}_{j}")
                        avp1 = avpp.tile([65, 512], F32, tag="avp1",
                                         name=f"avp1_{p}_{j}")
                    diag = i >= 4 * j
                    c0 = 128 * (i - 4 * j) if diag else 0
                    pt = ptp.tile([128, 2, 512], F32, tag="pt",
                                  name=f"pt{p}_{j}_{i}")
                    # attT: both heads concurrently (row halves of the PE
                    # array) into the two 512-col blocks of one 2-bank
                    # PSUM tile; ONE exp ACTIVATE covers both via a 3D AP.
                    nc.tensor.matmul(
                        pt[:, 0, c0:512],
                        kt[0:64, 128 * i:128 * (i + 1)],
                        qt[0:64, 512 * j + c0:512 * (j + 1)],
                        start=True, stop=True)
                    nc.tensor.matmul(
                        pt[:, 1, c0:512],
                        kt[64:128, 128 * i:128 * (i + 1)],
                        qt[64:128, 512 * j + c0:512 * (j + 1)],
                        start=True, stop=True)
                    et = etp.tile([128, 2, 512], BF16, tag="et",
                                  name=f"et{p}_{j}_{i}")
                    nc.scalar.activation(et[:, :, c0:512],
                                         pt[:, :, c0:512], EXP)
                    # foreign closure BEFORE the lagged AV: gives the
                    # previous unit's exp a few hundred ns to finish so the
                    # AV never stalls the PE queue on the ACT engine.
                    # In the second half, hold ~8 closures back so the PE
                    # stays busy (and HAM stays warm) through the final
                    # normalize chain.
                    if foreign and (uidx < 44 or len(foreign) > 8):
                        foreign.pop(0)()
                    if pending is not None:
                        do_av(*pending)
                    pending = (p, j, i, c0, et, avp0, avp1)
                    if prev_norm is not None:
                        normalize(*prev_norm)
                        prev_norm = None
                    if i == 4 * j + 3:
                        prev_norm = (p, j, avp0, avp1)
                    if diag:
                        # causal mask: zero the upper triangle of the
                        # 128-wide diag strip (both heads in one DVE op,
                        # hidden in the exp->AV lag slack). Issued LAST in
                        # the unit so it does not block the strict-FIFO
                        # DVE queue while its exp is still running.
                        nc.vector.tensor_mul(et[:, :, c0:c0 + 128],
                                             et[:, :, c0:c0 + 128],
                                             tri2[:])
                do_av(*pending)
                # drain the reserved (independent) closures BEFORE the
                # final normalize: its broadcast matmuls wait ~2us on a
                # reshape DMA, and anything issued after them is stuck
                # behind that wait in the strict-FIFO PE queue.
                while foreign:
                    foreign.pop(0)()
                normalize(*prev_norm)
                # E closures for j=3 (appended by the final normalize)
                while foreign:
                    foreign.pop(0)()

    nc.compile()
    return nc


def _pack8(a):
    """[C, F] f32 -> [128, KP*2*F] fp8 host layout [p, kp, s, f]."""
    Cd, F = a.shape
    return np.ascontiguousarray(
        a.reshape(KP, 2, 128, F).transpose(2, 0, 1, 3).reshape(128, -1)
    ).astype(NPF8)


def _shard_inputs(x, w_qkv, b_qkv, w_proj):
    scale = 1.0 / np.sqrt(D)   # 0.125, exact power of two
    in_maps = []
    k_idx = np.arange(128)[:, None]
    m_idx = np.arange(128)[None, :]
    tri = (k_idx <= m_idx).astype(np.float32)
    tri2 = np.concatenate([tri, tri], axis=1).astype(NPBF)
    for core in range(N_CORES):
        b, g = divmod(core, HPG)
        qs = slice(CG * g, CG * (g + 1))
        ks = slice(C + CG * g, C + CG * (g + 1))
        vs = slice(2 * C + CG * g, 2 * C + CG * (g + 1))
        wqk = np.concatenate([w_qkv[qs] * scale, w_qkv[ks]], axis=0).T
        bqk = np.concatenate([b_qkv[qs] * scale, b_qkv[ks]])[:, None]
        wv_base = w_qkv[vs].T          # [C, 256]
        wv = np.zeros((C, VW), np.float32)
        bv = np.zeros((1, VW), np.float32)
        for h in range(HPG):
            wv[:, 65 * h:65 * h + 64] = wv_base[:, 64 * h:64 * h + 64]
            bv[0, 65 * h:65 * h + 64] = b_qkv[vs][64 * h:64 * h + 64]
            bv[0, 65 * h + 64] = 1.0
        xb = np.ascontiguousarray(x[b].T)          # [C, T]
        xt8s = np.ascontiguousarray(
            xb[768:1024].reshape(2, 128, T).transpose(1, 0, 2)
            .reshape(128, 2 * T)).astype(NPF8)
        in_maps.append({
            "wqk8": _pack8(wqk * SC),
            "xt8s": xt8s,
            "xt": xb.astype(NPBF),
            "wv": wv.astype(NPBF),
            "bqk": np.ascontiguousarray(bqk, np.float32),
            "bvf": np.broadcast_to(bv, (128, VW)).astype(NPBF),
            "wp": np.ascontiguousarray(
                w_proj[:, CG * g:CG * (g + 1)].T).astype(NPBF),
            "tri2": tri2,
            "ones16": np.ones((1, 64), NPBF),
        })
    return in_maps


def kernel(x, w_qkv, b_qkv, w_proj, b_proj):
    x = np.asarray(x, np.float32)
    w_qkv = np.asarray(w_qkv, np.float32)
    b_qkv = np.asarray(b_qkv, np.float32)
    w_proj = np.asarray(w_proj, np.float32)
    b_proj = np.asarray(b_proj, np.float32)

    nc = build_nc()
    in_maps = _shard_inputs(x, w_qkv, b_qkv, w_proj)
    if _trace_flag[0]:
        _ensure_ntff_hook()
    res = run_bass_kernel_spmd(nc, in_maps, core_ids=list(range(N_CORES)),
                               trace=_trace_flag[0])
    _last_results[0] = res

    y = np.empty((B, T, C), np.float32)
    for b in range(B):
        acc = np.zeros((C, T), np.float32)
        for g in range(HPG):
            acc += np.asarray(res.results[HPG * b + g]["yt"], np.float32)
        y[b] = acc.T + b_proj[None, :]
    return y


# revision 35
# speedup vs baseline: 1.0444x; 1.0127x over previous
"""Causal self-attention (B=2, T=2048, C=1024, H=16, D=64) on 8 trn2 cores.

Sharding: core c -> batch b = c // 4, head-group g = c % 4 (4 heads each).
Data-parallel over B, tensor-parallel (Megatron) over heads for the
qkv / proj linears. Each core computes its head-group's attention and a
partial output projection; the host sums the 4 partials per batch and
adds the proj bias.

Structure (v1, from the ~161us baseline):
  * Inputs batched into ~11 DMA descriptors (descriptor issue on the Sync
    engine costs ~600ns each; the old ~30 descriptors serialized the
    startup). x is loaded ONCE in bf16; the fp8 copy for the q/k path is
    cast on-chip by the DVE (saves 2.1MB of HBM traffic on the critical
    startup path).
  * q/k projection (fp8 DoubleRow, weights pre-scaled x64 on host) runs
    kp-OUTER across 8 concurrent PSUM accumulation groups so the PE
    tracks the x DMA stream as chunk-pairs land.
  * v projection: first 4 row-chunks serial (k-inner), rest moved into
    the attention foreign queue.
  * Attention as 2 head-PAIRS, pt/et tiles laid out [128, 2, 512]
    (h0/h1 blocks); ONE exp ACTIVATE covers both heads via a 3D AP.
  * Causal mask: DVE multiply of the exp output's 128-wide diag strip by
    a constant 0/1 triangular [128, 2, 128] mask - off the PE entirely
    (the old identity-matmul mask cost ~417ns of LDW+MM per diag unit),
    hidden in the exp->AV lag slack.
  * Softmax denominator from a ones-column in v_aug (row 64 of the AV
    accumulator); reciprocal runs as [128,8] (DVE reciprocal costs
    ~6.5ns per free-column), reshaped via SBUF->SBUF DMA; broadcast
    across the 64 dim partitions via a DRAM round trip with
    partition-stride-0 read. The final block broadcasts on-chip (K=1
    matmul).
  * Stage B group B (q/k heads 2,3), v chunks 4-15 and the output
    projection are issued as closures interleaved one-per-unit into the
    attention loop to fill the ACT-bound PE gaps; ~6 closures are held
    back for the final normalize window so the PE (and its HAM clock)
    stays busy while the last block's reciprocal chain runs.
"""

import os
import sys
import types

for _p in ("/opt/trn_rl_repo", "/root/.axon_site", "/root/.axon_site/_ro/trn_rl_repo"):
    if os.path.isdir(_p) and _p not in sys.path:
        sys.path.append(_p)

import numpy as np
import ml_dtypes

import concourse.bacc as bacc
import concourse.bass as bass
import concourse.mybir as mybir
import concourse.tile as tile
from concourse.bass_utils import run_bass_kernel_spmd

# ── problem constants (hardcoded; spec.json not available at grade time) ──
B, T, C = 2, 2048, 1024
H, D = 16, 64
N_CORES = 8
HPG = 4                 # heads per group (per core)
CG = HPG * D            # 256 channels per head-group
NT = T // 512           # 4 query chunks of 512
KC = C // 128           # 8 contraction tiles for C
KP = KC // 2            # 4 fp8 DoubleRow K-pair chunks
VW = HPG * 65           # v tile width: 4x(64 dims + ones col) = 260
SC = 64.0               # fp8 weight pre-scale (power of two)

F32 = mybir.dt.float32
BF16 = mybir.dt.bfloat16
F8 = mybir.dt.float8e4
EXP = mybir.ActivationFunctionType.Exp
DR = mybir.MatmulPerfMode.DoubleRow
MULT = mybir.AluOpType.mult
ADD = mybir.AluOpType.add
NPBF = ml_dtypes.bfloat16
NPF8 = ml_dtypes.float8_e4m3fn

_trace_flag = [False]   # test.py can flip this to capture a profile
_last_results = [None]


def _ensure_ntff_hook():
    """Install the NTFF profile hook shim (container's antenv lacks it)."""
    if "antenv.axon_hooks" in sys.modules:
        return
    try:
        from trn_agent_boot.trn_boot import _ntff_profile_via_ctypes
    except Exception:
        return
    mod = types.ModuleType("antenv.axon_hooks")
    hook = [None]
    mod.set_axon_ntff_profile_hook = lambda h: hook.__setitem__(0, h)
    mod.get_axon_ntff_profile_hook = lambda: hook[0]
    sys.modules["antenv.axon_hooks"] = mod
    so = "/opt/axon/libaxon_pjrt.so"
    if os.path.exists(so):
        mod.set_axon_ntff_profile_hook(_ntff_profile_via_ctypes(so))


def build_nc():
    nc = bacc.Bacc("TRN2", target_bir_lowering=False, debug=False,
                   num_devices=N_CORES)

    # fp8 q/k weights, host-packed as [p, kp, s, f] -> [128, KP*2*2CG]
    wqk8_d = nc.dram_tensor("wqk8", [128, KP * 2 * 2 * CG], F8,
                            kind="ExternalInput").ap()
    # fp8 seed for the kp=3 chunk of x: the first DoubleRow matmul does not
    # wait for a bf16 chunk + cast, and the LAST-arriving bf16 chunk (xtp3)
    # is not on stage A's critical path; kp 0-2 are cast from xtp on-chip.
    xt8s_d = nc.dram_tensor("xt8s", [128, 2 * T], F8,
                            kind="ExternalInput").ap()
    xt_d = nc.dram_tensor("xt", [C, T], BF16, kind="ExternalInput").ap()
    wv_d = nc.dram_tensor("wv", [C, VW], BF16, kind="ExternalInput").ap()
    bqk_d = nc.dram_tensor("bqk", [2 * CG, 1], F32, kind="ExternalInput").ap()
    bvf_d = nc.dram_tensor("bvf", [128, VW], BF16, kind="ExternalInput").ap()
    wp_d = nc.dram_tensor("wp", [CG, C], BF16, kind="ExternalInput").ap()
    # tri2[k, s, m] = 1.0 where k <= m else 0 (causal diag strip mask)
    tri2_d = nc.dram_tensor("tri2", [128, 256], BF16, kind="ExternalInput").ap()
    ones16_d = nc.dram_tensor("ones16", [1, 64], BF16, kind="ExternalInput").ap()
    yt_d = nc.dram_tensor("yt", [C, T], BF16, kind="ExternalOutput").ap()
    rec_d = nc.dram_tensor("rec_scratch", [32, 512], F32).ap()

    with tile.TileContext(nc) as tc:
        with tc.tile_pool(name="const", bufs=1) as cp:
            # ── persistent SBUF residents ──
            wqk8 = cp.tile([128, KP, 2, 2 * CG], F8, tag="wqk8")
            xtp = [cp.tile([128, 2, T], BF16, tag=f"xtp{kp}", name=f"xtp{kp}")
                   for kp in range(KP)]
            xt8 = [cp.tile([128, 2, T], F8, tag=f"xt8{kp}", name=f"xt8{kp}")
                   for kp in range(KP)]
            warm = cp.tile([128, 128], BF16, tag="warm")
            wv = cp.tile([128, KC, VW], BF16, tag="wv")
            bqk = cp.tile([128, 4], F32, tag="bqk")
            bvf = cp.tile([128, VW], BF16, tag="bvf")
            wp = cp.tile([128, 2, C], BF16, tag="wp")
            tri2 = cp.tile([128, 2, 128], BF16, tag="tri2")
            ones16 = cp.tile([1, 64], BF16, tag="ones16")
            # qk[0]=q heads01, qk[1]=q heads23, qk[2]=k heads01, qk[3]=k heads23
            # (head pair laid out as rows 0-63 / 64-127 of the tile)
            qk = [cp.tile([128, T], BF16, tag=f"qk{m}", name=f"qk{m}")
                  for m in range(4)]
            v_sb = [cp.tile([128, VW], BF16, tag=f"v{m}", name=f"v{m}")
                    for m in range(T // 128)]
            outT = [cp.tile([128, T], BF16, tag=f"outT{k}", name=f"outT{k}")
                    for k in range(2)]

            # ── input DMA: few, large descriptors; x + q/k weights first ──
            nc.sync.dma_start(wqk8[:], wqk8_d[:, :])
            nc.sync.dma_start(xt8[3][:], xt8s_d[:])
            for kp in range(2):
                nc.sync.dma_start(
                    xtp[kp][:],
                    bass.AP(xt_d.tensor, 256 * T * kp,
                            [[T, 128], [128 * T, 2], [1, T]]))
            nc.sync.dma_start(
                wv[:], bass.AP(wv_d.tensor, 0,
                               [[VW, 128], [128 * VW, KC], [1, VW]]))
            for kp in range(2, KP):
                nc.sync.dma_start(
                    xtp[kp][:],
                    bass.AP(xt_d.tensor, 256 * T * kp,
                            [[T, 128], [128 * T, 2], [1, T]]))
            nc.sync.dma_start(
                bqk[:], bass.AP(bqk_d.tensor, 0, [[1, 128], [128, 4]]))
            nc.sync.dma_start(bvf[:], bvf_d[:])
            nc.sync.dma_start(tri2[:], tri2_d[:])
            nc.sync.dma_start(
                wp[:], bass.AP(wp_d.tensor, 0,
                               [[C, 128], [128 * C, 2], [1, C]]))
            nc.sync.dma_start(ones16[:], ones16_d[:])

            # ── HAM warmup: ~3.5us of dummy matmuls with no DMA deps so the
            # PE clock gate opens before the first real (DMA-gated) matmul ──
            nc.vector.memset(warm[:], 0)
            with tc.tile_pool(name="psW", bufs=1, space="PSUM") as psW:
                pw = psW.tile([128, 512], F32, tag="pw")
                for w in range(34):
                    nc.tensor.matmul(pw[:, 0:128], warm[:], warm[:],
                                     start=True, stop=True)

            # on-chip bf16 -> fp8 casts for the q/k DoubleRow path (kp 0-2),
            # split per T-half so stage A's nt 0-1 matmuls start one half-
            # cast earlier
            for kp in range(KP - 1):
                for s in range(2):
                    for h in range(2):
                        nc.vector.tensor_copy(
                            xt8[kp][:, s, 1024 * h:1024 * (h + 1)],
                            xtp[kp][:, s, 1024 * h:1024 * (h + 1)])

            # ── stage A: q,k for heads 0,1 (mf 0 and 2). kp-OUTER across 8
            # concurrent PSUM groups so the PE tracks the x DMA stream; the
            # fp8 seed chunk (kp=3) runs first with no cast dependency. ──
            with tc.tile_pool(name="psAC", bufs=1, space="PSUM") as psA:
                pA = {}
                for mf in (0, 2):
                    for nt in range(NT):
                        pA[(mf, nt)] = psA.tile([128, 512], F32,
                                                tag=f"pA{mf}_{nt}",
                                                name=f"pA{mf}_{nt}")
                for kpi, kp in enumerate((3, 0, 1, 2)):
                    for nt in range(NT):
                        for mf in (0, 2):
                            nc.tensor.matmul(
                                pA[(mf, nt)][:],
                                wqk8[:, kp, :, 128 * mf:128 * (mf + 1)],
                                xt8[kp][:, :, 512 * nt:512 * (nt + 1)],
                                start=(kpi == 0), stop=(kpi == KP - 1),
                                perf_mode=DR)

                # ── bias-adds + stage C part 1 (v chunks 0-3), interleaved
                # so attention's j=0 gates (qk nt=0, v_sb[0..3]) clear as
                # early as possible. psv(mt) reuses pA(0,mt)'s PSUM bank,
                # whose bias-add runs just before it. ──
                def bias(mf, nt):
                    nc.vector.tensor_scalar(
                        qk[mf][:, 512 * nt:512 * (nt + 1)],
                        pA[(mf, nt)][:], 1.0 / SC, bqk[:, mf:mf + 1],
                        op0=MULT, op1=ADD)

                bias(0, 0)
                bias(2, 0)
                for mt in range(4):
                    ps = psA.tile([128, 512], F32, tag=f"pA0_{mt}",
                                  name=f"psv{mt}")
                    for k in range(KC):
                        nc.tensor.matmul(
                            ps[:, 0:VW],
                            xtp[k // 2][:, k % 2, 128 * mt:128 * (mt + 1)],
                            wv[:, k, :], start=(k == 0), stop=(k == KC - 1))
                    nc.vector.scalar_tensor_tensor(
                        v_sb[mt][:], ps[:, 0:VW], 1.0, bvf[:],
                        op0=MULT, op1=ADD)
                    if mt < 3:
                        bias(0, mt + 1)
                for nt in range(1, NT):
                    bias(2, nt)

            # ── stage D: attention, two head-pairs. j outer, key chunk i
            # inner. Foreign PE work (stage B group B = q,k heads 2,3;
            # stage C tails; stage E output projection) interleaved one
            # closure per unit. ──
            with (
                tc.tile_pool(name="ptp", bufs=2, space="PSUM") as ptp,
                tc.tile_pool(name="avpp", bufs=1, space="PSUM") as avpp,
                tc.tile_pool(name="fxp", bufs=2, space="PSUM") as fxp,
                tc.tile_pool(name="etp", bufs=4) as etp,
                tc.tile_pool(name="rawp", bufs=2) as rawp,
                tc.tile_pool(name="recp", bufs=2) as recp,
                tc.tile_pool(name="bcp", bufs=2) as bcp,
                tc.tile_pool(name="otp", bufs=4) as otp,
            ):
                foreign = []

                # stage C tail closures: v chunks 4-15, two closures of 4
                # K-steps each.
                def mk_c(mt):
                    holder = {}
                    cls = []
                    for half in range(2):
                        def ccl(mt=mt, half=half, holder=holder):
                            if half == 0:
                                holder["ps"] = fxp.tile(
                                    [128, 512], F32, tag="fx",
                                    name=f"pc{mt}")
                            ps = holder["ps"]
                            for k in range(4 * half, 4 * half + 4):
                                nc.tensor.matmul(
                                    ps[:, 0:VW],
                                    xtp[k // 2][:, k % 2,
                                                128 * mt:128 * (mt + 1)],
                                    wv[:, k, :], start=(k == 0),
                                    stop=(k == KC - 1))
                            if half == 1:
                                nc.vector.scalar_tensor_tensor(
                                    v_sb[mt][:], ps[:, 0:VW], 1.0, bvf[:],
                                    op0=MULT, op1=ADD)
                        cls.append(ccl)
                    return cls

                for mt in range(4, 12):
                    foreign.extend(mk_c(mt))

                # stage B group B closures: mf 1 (q23) and 3 (k23), two
                # closures of 2 DoubleRow K-pair steps each.
                for mf in (1, 3):
                    for nt in range(NT):
                        holder = {}
                        for half in range(2):
                            def bgc(mf=mf, nt=nt, half=half, holder=holder):
                                if half == 0:
                                    holder["ps"] = fxp.tile(
                                        [128, 512], F32, tag="fx",
                                        name=f"bg{mf}_{nt}")
                                ps = holder["ps"]
                                for kp in (2 * half, 2 * half + 1):
                                    nc.tensor.matmul(
                                        ps[:],
                                        wqk8[:, kp, :,
                                             128 * mf:128 * (mf + 1)],
                                        xt8[kp][:, :,
                                                512 * nt:512 * (nt + 1)],
                                        start=(kp == 0), stop=(kp == KP - 1),
                                        perf_mode=DR)
                                if half == 1:
                                    nc.vector.tensor_scalar(
                                        qk[mf][:, 512 * nt:512 * (nt + 1)],
                                        ps[:], 1.0 / SC, bqk[:, mf:mf + 1],
                                        op0=MULT, op1=ADD)
                            foreign.append(bgc)
                for mt in range(12, T // 128):
                    foreign.extend(mk_c(mt))

                def mk_e(mo, nt):
                    def ecl():
                        ps = fxp.tile([128, 512], F32, tag="fx",
                                      name=f"pe{mo}_{nt}")
                        for k in range(2):
                            nc.tensor.matmul(
                                ps[:], wp[:, k, 128 * mo:128 * (mo + 1)],
                                outT[k][:, 512 * nt:512 * (nt + 1)],
                                start=(k == 0), stop=(k == 1))
                        ot = otp.tile([128, 512], BF16, tag="ot",
                                      name=f"ot{mo}_{nt}")
                        if nt == 3 and mo % 2 == 0:
                            # tail chunk: ACT is idle by then; alternate
                            # with DVE so neither queue paces the drain
                            nc.scalar.copy(ot[:], ps[:])
                        else:
                            nc.vector.tensor_copy(ot[:], ps[:])
                        nc.sync.dma_start(
                            yt_d[128 * mo:128 * (mo + 1),
                                 512 * nt:512 * (nt + 1)], ot[:])
                    return ecl

                def do_av(p, j, i, c0, et, avp0, avp1):
                    last = (i == 4 * j + 3)
                    nc.tensor.matmul(
                        avp0[:, c0:512],
                        v_sb[i][:, 130 * p:130 * p + 65],
                        et[:, 0, c0:512], start=(i == 0), stop=last)
                    nc.tensor.matmul(
                        avp1[:, c0:512],
                        v_sb[i][:, 130 * p + 65:130 * p + 130],
                        et[:, 1, c0:512],
                        start=(i == 0), stop=last)

                def normalize(p, j, avp0, avp1):
                    # DVE reciprocal costs ~6.5ns/free-column no matter
                    # the partition count, so reshape the 2x512 denoms
                    # to [128,8] via SBUF->SBUF DMA (26ns recip). Raw
                    # copies come first so both avp banks free
                    # immediately for the next j's accumulation.
                    u2 = 8 * p + 2 * j
                    raws = []
                    den2 = recp.tile([128, 8], F32, tag="den2",
                                     name=f"den2_{p}_{j}")
                    for hh in range(2):
                        avp = avp0 if hh == 0 else avp1
                        raw = rawp.tile([65, 512], F32, tag=f"raw{hh}",
                                        name=f"raw{p}_{hh}_{j}")
                        nc.vector.tensor_copy(raw[:], avp[:])
                        nc.sync.dma_start(den2[64 * hh:64 * hh + 64, :],
                                          raw[64:65, :])
                        raws.append(raw)
                    rec2 = recp.tile([128, 8], F32, tag="rec2",
                                     name=f"rec2_{p}_{j}")
                    nc.vector.reciprocal(rec2[:], den2[:])
                    nc.sync.dma_start(
                        bass.AP(rec_d.tensor, u2 * 512,
                                [[8, 128], [1, 8]]), rec2[:])
                    for hh in range(2):
                        bc = bcp.tile([64, 512], F32, tag=f"bc{hh}",
                                      name=f"bc{p}_{hh}_{j}")
                        nc.sync.dma_start(
                            bc[:], bass.AP(rec_d.tensor,
                                           (u2 + hh) * 512,
                                           [[0, 64], [1, 512]]))
                        nc.vector.tensor_mul(
                            outT[p][64 * hh:64 * hh + 64,
                                    512 * j:512 * (j + 1)],
                            raws[hh][0:64, :], bc[:])
                    if p == 1:
                        for mo in range(8):
                            foreign.append(mk_e(mo, j))

                def norm_final_head(p, j, avp0, avp1):
                    # final block, part 1: everything up to the rec_row
                    # reshape DMA — no PE instructions, so the reserved
                    # foreign closures drained right after this fill the
                    # PE while the DMA latency plays out. One raw copy on
                    # ACT (idle by now) so the pair runs in parallel.
                    raws = []
                    den2 = recp.tile([128, 8], F32, tag="den2",
                                     name="den2_f")
                    for hh in range(2):
                        avp = avp0 if hh == 0 else avp1
                        raw = rawp.tile([65, 512], F32, tag=f"raw{hh}",
                                        name=f"rawf_{hh}")
                        if hh == 1:
                            nc.scalar.copy(raw[:], avp[:])
                        else:
                            nc.vector.tensor_copy(raw[:], avp[:])
                        nc.sync.dma_start(den2[64 * hh:64 * hh + 64, :],
                                          raw[64:65, :])
                        raws.append(raw)
                    rec2 = recp.tile([128, 8], F32, tag="rec2",
                                     name="rec2_f")
                    nc.vector.reciprocal(rec2[:], den2[:])
                    rec2b = recp.tile([128, 8], BF16, tag="rec2b",
                                      name="rec2b")
                    nc.vector.tensor_copy(rec2b[:], rec2[:])
                    rec_row = recp.tile([1, 1024], BF16,
                                        tag="recrow", name="recrow")
                    nc.sync.dma_start(rec_row[:], rec2b[:])
                    return p, j, raws, rec_row

                def norm_final_tail(p, j, raws, rec_row):
                    # final block, part 2: on-chip K=1 PE broadcast of the
                    # reciprocals + the normalize multiplies, then the j=3
                    # output-projection closures.
                    for hh in range(2):
                        bcps = fxp.tile([128, 512], F32, tag="fx",
                                        name=f"bcps{hh}")
                        nc.tensor.matmul(
                            bcps[0:64, :], ones16[:],
                            rec_row[0:1, 512 * hh:512 * (hh + 1)],
                            start=True, stop=True)
                        nc.vector.tensor_mul(
                            outT[p][64 * hh:64 * hh + 64,
                                    512 * j:512 * (j + 1)],
                            raws[hh][0:64, :], bcps[0:64, :])
                    for mo in range(8):
                        foreign.append(mk_e(mo, j))

                # flattened unit stream, software-pipelined ACROSS j (and
                # pair) boundaries: the next block's attT+exp issue before
                # the previous block's last AV flush and normalize, so the
                # ACT engine never waits at a block boundary.
                units = [(p, j, i) for p in range(2) for j in range(NT)
                         for i in range(4 * j + 4)]
                n_units = len(units)
                pending = None
                prev_norm = None
                avp0 = avp1 = None
                for uidx, (p, j, i) in enumerate(units):
                    qt = qk[p]
                    kt = qk[2 + p]
                    if i == 0:
                        avp0 = avpp.tile([65, 512], F32, tag="avp0",
                                         name=f"avp0_{p}_{j}")
                        avp1 = avpp.tile([65, 512], F32, tag="avp1",
                                         name=f"avp1_{p}_{j}")
                    diag = i >= 4 * j
                    c0 = 128 * (i - 4 * j) if diag else 0
                    pt = ptp.tile([128, 2, 512], F32, tag="pt",
                                  name=f"pt{p}_{j}_{i}")
                    # attT: both heads concurrently (row halves of the PE
                    # array) into the two 512-col blocks of one 2-bank
                    # PSUM tile; ONE exp ACTIVATE covers both via a 3D AP.
                    nc.tensor.matmul(
                        pt[:, 0, c0:512],
                        kt[0:64, 128 * i:128 * (i + 1)],
                        qt[0:64, 512 * j + c0:512 * (j + 1)],
                        start=True, stop=True)
                    nc.tensor.matmul(
                        pt[:, 1, c0:512],
                        kt[64:128, 128 * i:128 * (i + 1)],
                        qt[64:128, 512 * j + c0:512 * (j + 1)],
                        start=True, stop=True)
                    et = etp.tile([128, 2, 512], BF16, tag="et",
                                  name=f"et{p}_{j}_{i}")
                    nc.scalar.activation(et[:, :, c0:512],
                                         pt[:, :, c0:512], EXP)
                    # foreign closure BEFORE the lagged AV: gives the
                    # previous unit's exp a few hundred ns to finish so the
                    # AV never stalls the PE queue on the ACT engine.
                    # In the second half, hold ~8 closures back so the PE
                    # stays busy (and HAM stays warm) through the final
                    # normalize chain.
                    if foreign and (uidx < 44 or len(foreign) > 8):
                        foreign.pop(0)()
                    if pending is not None:
                        do_av(*pending)
                    pending = (p, j, i, c0, et, avp0, avp1)
                    if prev_norm is not None:
                        normalize(*prev_norm)
                        prev_norm = None
                    if i == 4 * j + 3:
                        prev_norm = (p, j, avp0, avp1)
                    if diag:
                        # causal mask: zero the upper triangle of the
                        # 128-wide diag strip (both heads in one DVE op,
                        # hidden in the exp->AV lag slack). Issued LAST in
                        # the unit so it does not block the strict-FIFO
                        # DVE queue while its exp is still running.
                        nc.vector.tensor_mul(et[:, :, c0:c0 + 128],
                                             et[:, :, c0:c0 + 128],
                                             tri2[:])
                do_av(*pending)
                # final block: start the reciprocal chain (DVE/ACT/DMA
                # only), fill the PE with the reserved closures while its
                # DMA latency plays out, then the broadcast+muls and the
                # j=3 output projection.
                saved = norm_final_head(*prev_norm)
                while foreign:
                    foreign.pop(0)()
                norm_final_tail(*saved)
                while foreign:
                    foreign.pop(0)()

    nc.compile()
    return nc


def _pack8(a):
    """[C, F] f32 -> [128, KP*2*F] fp8 host layout [p, kp, s, f]."""
    Cd, F = a.shape
    return np.ascontiguousarray(
        a.reshape(KP, 2, 128, F).transpose(2, 0, 1, 3).reshape(128, -1)
    ).astype(NPF8)


def _shard_inputs(x, w_qkv, b_qkv, w_proj):
    scale = 1.0 / np.sqrt(D)   # 0.125, exact power of two
    in_maps = []
    k_idx = np.arange(128)[:, None]
    m_idx = np.arange(128)[None, :]
    tri = (k_idx <= m_idx).astype(np.float32)
    tri2 = np.concatenate([tri, tri], axis=1).astype(NPBF)
    for core in range(N_CORES):
        b, g = divmod(core, HPG)
        qs = slice(CG * g, CG * (g + 1))
        ks = slice(C + CG * g, C + CG * (g + 1))
        vs = slice(2 * C + CG * g, 2 * C + CG * (g + 1))
        wqk = np.concatenate([w_qkv[qs] * scale, w_qkv[ks]], axis=0).T
        bqk = np.concatenate([b_qkv[qs] * scale, b_qkv[ks]])[:, None]
        wv_base = w_qkv[vs].T          # [C, 256]
        wv = np.zeros((C, VW), np.float32)
        bv = np.zeros((1, VW), np.float32)
        for h in range(HPG):
            wv[:, 65 * h:65 * h + 64] = wv_base[:, 64 * h:64 * h + 64]
            bv[0, 65 * h:65 * h + 64] = b_qkv[vs][64 * h:64 * h + 64]
            bv[0, 65 * h + 64] = 1.0
        xb = np.ascontiguousarray(x[b].T)          # [C, T]
        xt8s = np.ascontiguousarray(
            xb[768:1024].reshape(2, 128, T).transpose(1, 0, 2)
            .reshape(128, 2 * T)).astype(NPF8)
        in_maps.append({
            "wqk8": _pack8(wqk * SC),
            "xt8s": xt8s,
            "xt": xb.astype(NPBF),
            "wv": wv.astype(NPBF),
            "bqk": np.ascontiguousarray(bqk, np.float32),
            "bvf": np.broadcast_to(bv, (128, VW)).astype(NPBF),
            "wp": np.ascontiguousarray(
                w_proj[:, CG * g:CG * (g + 1)].T).astype(NPBF),
            "tri2": tri2,
            "ones16": np.ones((1, 64), NPBF),
        })
    return in_maps


def kernel(x, w_qkv, b_qkv, w_proj, b_proj):
    x = np.asarray(x, np.float32)
    w_qkv = np.asarray(w_qkv, np.float32)
    b_qkv = np.asarray(b_qkv, np.float32)
    w_proj = np.asarray(w_proj, np.float32)
    b_proj = np.asarray(b_proj, np.float32)

    nc = build_nc()
    in_maps = _shard_inputs(x, w_qkv, b_qkv, w_proj)
    if _trace_flag[0]:
        _ensure_ntff_hook()
    res = run_bass_kernel_spmd(nc, in_maps, core_ids=list(range(N_CORES)),
                               trace=_trace_flag[0])
    _last_results[0] = res

    y = np.empty((B, T, C), np.float32)
    for b in range(B):
        acc = np.zeros((C, T), np.float32)
        for g in range(HPG):
            acc += np.asarray(res.results[HPG * b + g]["yt"], np.float32)
        y[b] = acc.T + b_proj[None, :]
    return y


# revision 36
# speedup vs baseline: 1.0535x; 1.0088x over previous
"""Causal self-attention (B=2, T=2048, C=1024, H=16, D=64) on 8 trn2 cores.

Sharding: core c -> batch b = c // 4, head-group g = c % 4 (4 heads each).
Data-parallel over B, tensor-parallel (Megatron) over heads for the
qkv / proj linears. Each core computes its head-group's attention and a
partial output projection; the host sums the 4 partials per batch and
adds the proj bias.

Structure (v1, from the ~161us baseline):
  * Inputs batched into ~11 DMA descriptors (descriptor issue on the Sync
    engine costs ~600ns each; the old ~30 descriptors serialized the
    startup). x is loaded ONCE in bf16; the fp8 copy for the q/k path is
    cast on-chip by the DVE (saves 2.1MB of HBM traffic on the critical
    startup path).
  * q/k projection (fp8 DoubleRow, weights pre-scaled x64 on host) runs
    kp-OUTER across 8 concurrent PSUM accumulation groups so the PE
    tracks the x DMA stream as chunk-pairs land.
  * v projection: first 4 row-chunks serial (k-inner), rest moved into
    the attention foreign queue.
  * Attention as 2 head-PAIRS, pt/et tiles laid out [128, 2, 512]
    (h0/h1 blocks); ONE exp ACTIVATE covers both heads via a 3D AP.
  * Causal mask: DVE multiply of the exp output's 128-wide diag strip by
    a constant 0/1 triangular [128, 2, 128] mask - off the PE entirely
    (the old identity-matmul mask cost ~417ns of LDW+MM per diag unit),
    hidden in the exp->AV lag slack.
  * Softmax denominator from a ones-column in v_aug (row 64 of the AV
    accumulator); reciprocal runs as [128,8] (DVE reciprocal costs
    ~6.5ns per free-column), reshaped via SBUF->SBUF DMA; broadcast
    across the 64 dim partitions via a DRAM round trip with
    partition-stride-0 read. The final block broadcasts on-chip (K=1
    matmul).
  * Stage B group B (q/k heads 2,3), v chunks 4-15 and the output
    projection are issued as closures interleaved one-per-unit into the
    attention loop to fill the ACT-bound PE gaps; ~6 closures are held
    back for the final normalize window so the PE (and its HAM clock)
    stays busy while the last block's reciprocal chain runs.
"""

import os
import sys
import types

for _p in ("/opt/trn_rl_repo", "/root/.axon_site", "/root/.axon_site/_ro/trn_rl_repo"):
    if os.path.isdir(_p) and _p not in sys.path:
        sys.path.append(_p)

import numpy as np
import ml_dtypes

import concourse.bacc as bacc
import concourse.bass as bass
import concourse.mybir as mybir
import concourse.tile as tile
from concourse.bass_utils import run_bass_kernel_spmd

# ── problem constants (hardcoded; spec.json not available at grade time) ──
B, T, C = 2, 2048, 1024
H, D = 16, 64
N_CORES = 8
HPG = 4                 # heads per group (per core)
CG = HPG * D            # 256 channels per head-group
NT = T // 512           # 4 query chunks of 512
KC = C // 128           # 8 contraction tiles for C
KP = KC // 2            # 4 fp8 DoubleRow K-pair chunks
VW = HPG * 65           # v tile width: 4x(64 dims + ones col) = 260
SC = 64.0               # fp8 weight pre-scale (power of two)

F32 = mybir.dt.float32
BF16 = mybir.dt.bfloat16
F8 = mybir.dt.float8e4
EXP = mybir.ActivationFunctionType.Exp
DR = mybir.MatmulPerfMode.DoubleRow
MULT = mybir.AluOpType.mult
ADD = mybir.AluOpType.add
NPBF = ml_dtypes.bfloat16
NPF8 = ml_dtypes.float8_e4m3fn

_trace_flag = [False]   # test.py can flip this to capture a profile
_last_results = [None]


def _ensure_ntff_hook():
    """Install the NTFF profile hook shim (container's antenv lacks it)."""
    if "antenv.axon_hooks" in sys.modules:
        return
    try:
        from trn_agent_boot.trn_boot import _ntff_profile_via_ctypes
    except Exception:
        return
    mod = types.ModuleType("antenv.axon_hooks")
    hook = [None]
    mod.set_axon_ntff_profile_hook = lambda h: hook.__setitem__(0, h)
    mod.get_axon_ntff_profile_hook = lambda: hook[0]
    sys.modules["antenv.axon_hooks"] = mod
    so = "/opt/axon/libaxon_pjrt.so"
    if os.path.exists(so):
        mod.set_axon_ntff_profile_hook(_ntff_profile_via_ctypes(so))


def build_nc():
    nc = bacc.Bacc("TRN2", target_bir_lowering=False, debug=False,
                   num_devices=N_CORES)

    # fp8 q/k weights, host-packed as [p, kp, s, f] -> [128, KP*2*2CG]
    wqk8_d = nc.dram_tensor("wqk8", [128, KP * 2 * 2 * CG], F8,
                            kind="ExternalInput").ap()
    # fp8 seed for the kp=3 chunk of x: the first DoubleRow matmul does not
    # wait for a bf16 chunk + cast, and the LAST-arriving bf16 chunk (xtp3)
    # is not on stage A's critical path; kp 0-2 are cast from xtp on-chip.
    xt8s_d = nc.dram_tensor("xt8s", [128, 2 * 2 * T], F8,
                            kind="ExternalInput").ap()
    xt_d = nc.dram_tensor("xt", [C, T], BF16, kind="ExternalInput").ap()
    wv_d = nc.dram_tensor("wv", [C, VW], BF16, kind="ExternalInput").ap()
    bqk_d = nc.dram_tensor("bqk", [2 * CG, 1], F32, kind="ExternalInput").ap()
    bvf_d = nc.dram_tensor("bvf", [128, VW], BF16, kind="ExternalInput").ap()
    wp_d = nc.dram_tensor("wp", [CG, C], BF16, kind="ExternalInput").ap()
    # tri2[k, s, m] = 1.0 where k <= m else 0 (causal diag strip mask)
    tri2_d = nc.dram_tensor("tri2", [128, 256], BF16, kind="ExternalInput").ap()
    ones16_d = nc.dram_tensor("ones16", [1, 64], BF16, kind="ExternalInput").ap()
    yt_d = nc.dram_tensor("yt", [C, T], BF16, kind="ExternalOutput").ap()
    rec_d = nc.dram_tensor("rec_scratch", [32, 512], F32).ap()

    with tile.TileContext(nc) as tc:
        with tc.tile_pool(name="const", bufs=1) as cp:
            # ── persistent SBUF residents ──
            wqk8 = cp.tile([128, KP, 2, 2 * CG], F8, tag="wqk8")
            xtp = [cp.tile([128, 2, T], BF16, tag=f"xtp{kp}", name=f"xtp{kp}")
                   for kp in range(KP)]
            xt8 = [cp.tile([128, 2, T], F8, tag=f"xt8{kp}", name=f"xt8{kp}")
                   for kp in range(KP)]
            warm = cp.tile([128, 128], BF16, tag="warm")
            wv = cp.tile([128, KC, VW], BF16, tag="wv")
            bqk = cp.tile([128, 4], F32, tag="bqk")
            bvf = cp.tile([128, VW], BF16, tag="bvf")
            wp = cp.tile([128, 2, C], BF16, tag="wp")
            tri2 = cp.tile([128, 2, 128], BF16, tag="tri2")
            ones16 = cp.tile([1, 64], BF16, tag="ones16")
            # qk[0]=q heads01, qk[1]=q heads23, qk[2]=k heads01, qk[3]=k heads23
            # (head pair laid out as rows 0-63 / 64-127 of the tile)
            qk = [cp.tile([128, T], BF16, tag=f"qk{m}", name=f"qk{m}")
                  for m in range(4)]
            v_sb = [cp.tile([128, VW], BF16, tag=f"v{m}", name=f"v{m}")
                    for m in range(T // 128)]
            outT = [cp.tile([128, T], BF16, tag=f"outT{k}", name=f"outT{k}")
                    for k in range(2)]

            # ── input DMA: few, large descriptors; x + q/k weights first ──
            nc.sync.dma_start(wqk8[:], wqk8_d[:, :])
            nc.sync.dma_start(xt8[2][:], xt8s_d[:, 0:2 * T])
            nc.sync.dma_start(xt8[3][:], xt8s_d[:, 2 * T:4 * T])
            for kp in range(2):
                nc.sync.dma_start(
                    xtp[kp][:],
                    bass.AP(xt_d.tensor, 256 * T * kp,
                            [[T, 128], [128 * T, 2], [1, T]]))
            nc.sync.dma_start(
                wv[:], bass.AP(wv_d.tensor, 0,
                               [[VW, 128], [128 * VW, KC], [1, VW]]))
            for kp in range(2, KP):
                nc.sync.dma_start(
                    xtp[kp][:],
                    bass.AP(xt_d.tensor, 256 * T * kp,
                            [[T, 128], [128 * T, 2], [1, T]]))
            nc.sync.dma_start(
                bqk[:], bass.AP(bqk_d.tensor, 0, [[1, 128], [128, 4]]))
            nc.sync.dma_start(bvf[:], bvf_d[:])
            nc.sync.dma_start(tri2[:], tri2_d[:])
            nc.sync.dma_start(
                wp[:], bass.AP(wp_d.tensor, 0,
                               [[C, 128], [128 * C, 2], [1, C]]))
            nc.sync.dma_start(ones16[:], ones16_d[:])

            # ── HAM warmup: ~3.5us of dummy matmuls with no DMA deps so the
            # PE clock gate opens before the first real (DMA-gated) matmul ──
            nc.vector.memset(warm[:], 0)
            with tc.tile_pool(name="psW", bufs=1, space="PSUM") as psW:
                pw = psW.tile([128, 512], F32, tag="pw")
                for w in range(34):
                    nc.tensor.matmul(pw[:, 0:128], warm[:], warm[:],
                                     start=True, stop=True)

            # on-chip bf16 -> fp8 casts for the q/k DoubleRow path (kp 0-2),
            # split per T-half so stage A's nt 0-1 matmuls start one half-
            # cast earlier
            for kp in range(2):
                for s in range(2):
                    for h in range(2):
                        nc.vector.tensor_copy(
                            xt8[kp][:, s, 1024 * h:1024 * (h + 1)],
                            xtp[kp][:, s, 1024 * h:1024 * (h + 1)])

            # ── stage A: q,k for heads 0,1 (mf 0 and 2). kp-OUTER across 8
            # concurrent PSUM groups so the PE tracks the x DMA stream; the
            # fp8 seed chunk (kp=3) runs first with no cast dependency. ──
            with tc.tile_pool(name="psAC", bufs=1, space="PSUM") as psA:
                pA = {}
                for mf in (0, 2):
                    for nt in range(NT):
                        pA[(mf, nt)] = psA.tile([128, 512], F32,
                                                tag=f"pA{mf}_{nt}",
                                                name=f"pA{mf}_{nt}")
                for kpi, kp in enumerate((2, 3, 0, 1)):
                    for nt in range(NT):
                        for mf in (0, 2):
                            nc.tensor.matmul(
                                pA[(mf, nt)][:],
                                wqk8[:, kp, :, 128 * mf:128 * (mf + 1)],
                                xt8[kp][:, :, 512 * nt:512 * (nt + 1)],
                                start=(kpi == 0), stop=(kpi == KP - 1),
                                perf_mode=DR)

                # ── bias-adds + stage C part 1 (v chunks 0-3), interleaved
                # so attention's j=0 gates (qk nt=0, v_sb[0..3]) clear as
                # early as possible. psv(mt) reuses pA(0,mt)'s PSUM bank,
                # whose bias-add runs just before it. ──
                def bias(mf, nt):
                    nc.vector.tensor_scalar(
                        qk[mf][:, 512 * nt:512 * (nt + 1)],
                        pA[(mf, nt)][:], 1.0 / SC, bqk[:, mf:mf + 1],
                        op0=MULT, op1=ADD)

                bias(0, 0)
                bias(2, 0)
                for mt in range(4):
                    ps = psA.tile([128, 512], F32, tag=f"pA0_{mt}",
                                  name=f"psv{mt}")
                    for k in range(KC):
                        nc.tensor.matmul(
                            ps[:, 0:VW],
                            xtp[k // 2][:, k % 2, 128 * mt:128 * (mt + 1)],
                            wv[:, k, :], start=(k == 0), stop=(k == KC - 1))
                    nc.vector.scalar_tensor_tensor(
                        v_sb[mt][:], ps[:, 0:VW], 1.0, bvf[:],
                        op0=MULT, op1=ADD)
                    if mt < 3:
                        bias(0, mt + 1)
                for nt in range(1, NT):
                    bias(2, nt)

            # ── stage D: attention, two head-pairs. j outer, key chunk i
            # inner. Foreign PE work (stage B group B = q,k heads 2,3;
            # stage C tails; stage E output projection) interleaved one
            # closure per unit. ──
            with (
                tc.tile_pool(name="ptp", bufs=2, space="PSUM") as ptp,
                tc.tile_pool(name="avpp", bufs=1, space="PSUM") as avpp,
                tc.tile_pool(name="fxp", bufs=2, space="PSUM") as fxp,
                tc.tile_pool(name="etp", bufs=4) as etp,
                tc.tile_pool(name="rawp", bufs=2) as rawp,
                tc.tile_pool(name="recp", bufs=2) as recp,
                tc.tile_pool(name="bcp", bufs=2) as bcp,
                tc.tile_pool(name="otp", bufs=4) as otp,
            ):
                foreign = []

                # stage C tail closures: v chunks 4-15, two closures of 4
                # K-steps each.
                def mk_c(mt):
                    holder = {}
                    cls = []
                    for half in range(2):
                        def ccl(mt=mt, half=half, holder=holder):
                            if half == 0:
                                holder["ps"] = fxp.tile(
                                    [128, 512], F32, tag="fx",
                                    name=f"pc{mt}")
                            ps = holder["ps"]
                            for k in range(4 * half, 4 * half + 4):
                                nc.tensor.matmul(
                                    ps[:, 0:VW],
                                    xtp[k // 2][:, k % 2,
                                                128 * mt:128 * (mt + 1)],
                                    wv[:, k, :], start=(k == 0),
                                    stop=(k == KC - 1))
                            if half == 1:
                                nc.vector.scalar_tensor_tensor(
                                    v_sb[mt][:], ps[:, 0:VW], 1.0, bvf[:],
                                    op0=MULT, op1=ADD)
                        cls.append(ccl)
                    return cls

                for mt in range(4, 12):
                    foreign.extend(mk_c(mt))

                # stage B group B closures: mf 1 (q23) and 3 (k23), two
                # closures of 2 DoubleRow K-pair steps each.
                for mf in (1, 3):
                    for nt in range(NT):
                        holder = {}
                        for half in range(2):
                            def bgc(mf=mf, nt=nt, half=half, holder=holder):
                                if half == 0:
                                    holder["ps"] = fxp.tile(
                                        [128, 512], F32, tag="fx",
                                        name=f"bg{mf}_{nt}")
                                ps = holder["ps"]
                                for kp in (2 * half, 2 * half + 1):
                                    nc.tensor.matmul(
                                        ps[:],
                                        wqk8[:, kp, :,
                                             128 * mf:128 * (mf + 1)],
                                        xt8[kp][:, :,
                                                512 * nt:512 * (nt + 1)],
                                        start=(kp == 0), stop=(kp == KP - 1),
                                        perf_mode=DR)
                                if half == 1:
                                    nc.vector.tensor_scalar(
                                        qk[mf][:, 512 * nt:512 * (nt + 1)],
                                        ps[:], 1.0 / SC, bqk[:, mf:mf + 1],
                                        op0=MULT, op1=ADD)
                            foreign.append(bgc)
                for mt in range(12, T // 128):
                    foreign.extend(mk_c(mt))

                def mk_e(mo, nt):
                    def ecl():
                        ps = fxp.tile([128, 512], F32, tag="fx",
                                      name=f"pe{mo}_{nt}")
                        for k in range(2):
                            nc.tensor.matmul(
                                ps[:], wp[:, k, 128 * mo:128 * (mo + 1)],
                                outT[k][:, 512 * nt:512 * (nt + 1)],
                                start=(k == 0), stop=(k == 1))
                        ot = otp.tile([128, 512], BF16, tag="ot",
                                      name=f"ot{mo}_{nt}")
                        if nt == 3 and mo % 2 == 0:
                            # tail chunk: ACT is idle by then; alternate
                            # with DVE so neither queue paces the drain
                            nc.scalar.copy(ot[:], ps[:])
                        else:
                            nc.vector.tensor_copy(ot[:], ps[:])
                        nc.sync.dma_start(
                            yt_d[128 * mo:128 * (mo + 1),
                                 512 * nt:512 * (nt + 1)], ot[:])
                    return ecl

                def do_av(p, j, i, c0, et, avp0, avp1):
                    last = (i == 4 * j + 3)
                    nc.tensor.matmul(
                        avp0[:, c0:512],
                        v_sb[i][:, 130 * p:130 * p + 65],
                        et[:, 0, c0:512], start=(i == 0), stop=last)
                    nc.tensor.matmul(
                        avp1[:, c0:512],
                        v_sb[i][:, 130 * p + 65:130 * p + 130],
                        et[:, 1, c0:512],
                        start=(i == 0), stop=last)

                def normalize(p, j, avp0, avp1):
                    # DVE reciprocal costs ~6.5ns/free-column no matter
                    # the partition count, so reshape the 2x512 denoms
                    # to [128,8] via SBUF->SBUF DMA (26ns recip). Raw
                    # copies come first so both avp banks free
                    # immediately for the next j's accumulation.
                    u2 = 8 * p + 2 * j
                    raws = []
                    den2 = recp.tile([128, 8], F32, tag="den2",
                                     name=f"den2_{p}_{j}")
                    for hh in range(2):
                        avp = avp0 if hh == 0 else avp1
                        raw = rawp.tile([65, 512], F32, tag=f"raw{hh}",
                                        name=f"raw{p}_{hh}_{j}")
                        nc.vector.tensor_copy(raw[:], avp[:])
                        nc.sync.dma_start(den2[64 * hh:64 * hh + 64, :],
                                          raw[64:65, :])
                        raws.append(raw)
                    rec2 = recp.tile([128, 8], F32, tag="rec2",
                                     name=f"rec2_{p}_{j}")
                    nc.vector.reciprocal(rec2[:], den2[:])
                    nc.sync.dma_start(
                        bass.AP(rec_d.tensor, u2 * 512,
                                [[8, 128], [1, 8]]), rec2[:])
                    for hh in range(2):
                        bc = bcp.tile([64, 512], F32, tag=f"bc{hh}",
                                      name=f"bc{p}_{hh}_{j}")
                        nc.sync.dma_start(
                            bc[:], bass.AP(rec_d.tensor,
                                           (u2 + hh) * 512,
                                           [[0, 64], [1, 512]]))
                        nc.vector.tensor_mul(
                            outT[p][64 * hh:64 * hh + 64,
                                    512 * j:512 * (j + 1)],
                            raws[hh][0:64, :], bc[:])
                    if p == 1:
                        for mo in range(8):
                            foreign.append(mk_e(mo, j))

                def norm_final_head(p, j, avp0, avp1):
                    # final block, part 1: everything up to the rec_row
                    # reshape DMA — no PE instructions, so the reserved
                    # foreign closures drained right after this fill the
                    # PE while the DMA latency plays out. One raw copy on
                    # ACT (idle by now) so the pair runs in parallel.
                    raws = []
                    den2 = recp.tile([128, 8], F32, tag="den2",
                                     name="den2_f")
                    for hh in range(2):
                        avp = avp0 if hh == 0 else avp1
                        raw = rawp.tile([65, 512], F32, tag=f"raw{hh}",
                                        name=f"rawf_{hh}")
                        if hh == 1:
                            nc.scalar.copy(raw[:], avp[:])
                        else:
                            nc.vector.tensor_copy(raw[:], avp[:])
                        nc.sync.dma_start(den2[64 * hh:64 * hh + 64, :],
                                          raw[64:65, :])
                        raws.append(raw)
                    rec2 = recp.tile([128, 8], F32, tag="rec2",
                                     name="rec2_f")
                    nc.vector.reciprocal(rec2[:], den2[:])
                    rec2b = recp.tile([128, 8], BF16, tag="rec2b",
                                      name="rec2b")
                    nc.vector.tensor_copy(rec2b[:], rec2[:])
                    rec_row = recp.tile([1, 1024], BF16,
                                        tag="recrow", name="recrow")
                    nc.sync.dma_start(rec_row[:], rec2b[:])
                    return p, j, raws, rec_row

                def norm_final_tail(p, j, raws, rec_row):
                    # final block, part 2: on-chip K=1 PE broadcast of the
                    # reciprocals + the normalize multiplies, then the j=3
                    # output-projection closures.
                    for hh in range(2):
                        bcps = fxp.tile([128, 512], F32, tag="fx",
                                        name=f"bcps{hh}")
                        nc.tensor.matmul(
                            bcps[0:64, :], ones16[:],
                            rec_row[0:1, 512 * hh:512 * (hh + 1)],
                            start=True, stop=True)
                        nc.vector.tensor_mul(
                            outT[p][64 * hh:64 * hh + 64,
                                    512 * j:512 * (j + 1)],
                            raws[hh][0:64, :], bcps[0:64, :])
                    for mo in range(8):
                        foreign.append(mk_e(mo, j))

                # flattened unit stream, software-pipelined ACROSS j (and
                # pair) boundaries: the next block's attT+exp issue before
                # the previous block's last AV flush and normalize, so the
                # ACT engine never waits at a block boundary.
                units = [(p, j, i) for p in range(2) for j in range(NT)
                         for i in range(4 * j + 4)]
                n_units = len(units)
                pending = None
                prev_norm = None
                avp0 = avp1 = None
                for uidx, (p, j, i) in enumerate(units):
                    qt = qk[p]
                    kt = qk[2 + p]
                    if i == 0:
                        avp0 = avpp.tile([65, 512], F32, tag="avp0",
                                         name=f"avp0_{p}_{j}")
                        avp1 = avpp.tile([65, 512], F32, tag="avp1",
                                         name=f"avp1_{p}_{j}")
                    diag = i >= 4 * j
                    c0 = 128 * (i - 4 * j) if diag else 0
                    pt = ptp.tile([128, 2, 512], F32, tag="pt",
                                  name=f"pt{p}_{j}_{i}")
                    # attT: both heads concurrently (row halves of the PE
                    # array) into the two 512-col blocks of one 2-bank
                    # PSUM tile; ONE exp ACTIVATE covers both via a 3D AP.
                    nc.tensor.matmul(
                        pt[:, 0, c0:512],
                        kt[0:64, 128 * i:128 * (i + 1)],
                        qt[0:64, 512 * j + c0:512 * (j + 1)],
                        start=True, stop=True)
                    nc.tensor.matmul(
                        pt[:, 1, c0:512],
                        kt[64:128, 128 * i:128 * (i + 1)],
                        qt[64:128, 512 * j + c0:512 * (j + 1)],
                        start=True, stop=True)
                    et = etp.tile([128, 2, 512], BF16, tag="et",
                                  name=f"et{p}_{j}_{i}")
                    nc.scalar.activation(et[:, :, c0:512],
                                         pt[:, :, c0:512], EXP)
                    # foreign closure BEFORE the lagged AV: gives the
                    # previous unit's exp a few hundred ns to finish so the
                    # AV never stalls the PE queue on the ACT engine.
                    # In the second half, hold ~8 closures back so the PE
                    # stays busy (and HAM stays warm) through the final
                    # normalize chain.
                    if foreign and (uidx < 44 or len(foreign) > 8):
                        foreign.pop(0)()
                    if pending is not None:
                        do_av(*pending)
                    pending = (p, j, i, c0, et, avp0, avp1)
                    if prev_norm is not None:
                        normalize(*prev_norm)
                        prev_norm = None
                    if i == 4 * j + 3:
                        prev_norm = (p, j, avp0, avp1)
                    if diag:
                        # causal mask: zero the upper triangle of the
                        # 128-wide diag strip (both heads in one DVE op,
                        # hidden in the exp->AV lag slack). Issued LAST in
                        # the unit so it does not block the strict-FIFO
                        # DVE queue while its exp is still running.
                        nc.vector.tensor_mul(et[:, :, c0:c0 + 128],
                                             et[:, :, c0:c0 + 128],
                                             tri2[:])
                do_av(*pending)
                # final block: start the reciprocal chain (DVE/ACT/DMA
                # only), fill the PE with the reserved closures while its
                # DMA latency plays out, then the broadcast+muls and the
                # j=3 output projection.
                saved = norm_final_head(*prev_norm)
                while foreign:
                    foreign.pop(0)()
                norm_final_tail(*saved)
                while foreign:
                    foreign.pop(0)()

    nc.compile()
    return nc


def _pack8(a):
    """[C, F] f32 -> [128, KP*2*F] fp8 host layout [p, kp, s, f]."""
    Cd, F = a.shape
    return np.ascontiguousarray(
        a.reshape(KP, 2, 128, F).transpose(2, 0, 1, 3).reshape(128, -1)
    ).astype(NPF8)


def _shard_inputs(x, w_qkv, b_qkv, w_proj):
    scale = 1.0 / np.sqrt(D)   # 0.125, exact power of two
    in_maps = []
    k_idx = np.arange(128)[:, None]
    m_idx = np.arange(128)[None, :]
    tri = (k_idx <= m_idx).astype(np.float32)
    tri2 = np.concatenate([tri, tri], axis=1).astype(NPBF)
    for core in range(N_CORES):
        b, g = divmod(core, HPG)
        qs = slice(CG * g, CG * (g + 1))
        ks = slice(C + CG * g, C + CG * (g + 1))
        vs = slice(2 * C + CG * g, 2 * C + CG * (g + 1))
        wqk = np.concatenate([w_qkv[qs] * scale, w_qkv[ks]], axis=0).T
        bqk = np.concatenate([b_qkv[qs] * scale, b_qkv[ks]])[:, None]
        wv_base = w_qkv[vs].T          # [C, 256]
        wv = np.zeros((C, VW), np.float32)
        bv = np.zeros((1, VW), np.float32)
        for h in range(HPG):
            wv[:, 65 * h:65 * h + 64] = wv_base[:, 64 * h:64 * h + 64]
            bv[0, 65 * h:65 * h + 64] = b_qkv[vs][64 * h:64 * h + 64]
            bv[0, 65 * h + 64] = 1.0
        xb = np.ascontiguousarray(x[b].T)          # [C, T]
        xt8s = np.ascontiguousarray(
            xb[512:1024].reshape(2, 2, 128, T).transpose(2, 0, 1, 3)
            .reshape(128, 4 * T)).astype(NPF8)
        in_maps.append({
            "wqk8": _pack8(wqk * SC),
            "xt8s": xt8s,
            "xt": xb.astype(NPBF),
            "wv": wv.astype(NPBF),
            "bqk": np.ascontiguousarray(bqk, np.float32),
            "bvf": np.broadcast_to(bv, (128, VW)).astype(NPBF),
            "wp": np.ascontiguousarray(
                w_proj[:, CG * g:CG * (g + 1)].T).astype(NPBF),
            "tri2": tri2,
            "ones16": np.ones((1, 64), NPBF),
        })
    return in_maps


def kernel(x, w_qkv, b_qkv, w_proj, b_proj):
    x = np.asarray(x, np.float32)
    w_qkv = np.asarray(w_qkv, np.float32)
    b_qkv = np.asarray(b_qkv, np.float32)
    w_proj = np.asarray(w_proj, np.float32)
    b_proj = np.asarray(b_proj, np.float32)

    nc = build_nc()
    in_maps = _shard_inputs(x, w_qkv, b_qkv, w_proj)
    if _trace_flag[0]:
        _ensure_ntff_hook()
    res = run_bass_kernel_spmd(nc, in_maps, core_ids=list(range(N_CORES)),
                               trace=_trace_flag[0])
    _last_results[0] = res

    y = np.empty((B, T, C), np.float32)
    for b in range(B):
        acc = np.zeros((C, T), np.float32)
        for g in range(HPG):
            acc += np.asarray(res.results[HPG * b + g]["yt"], np.float32)
        y[b] = acc.T + b_proj[None, :]
    return y


# revision 39
# speedup vs baseline: 1.0839x; 1.0289x over previous
"""Causal self-attention (B=2, T=2048, C=1024, H=16, D=64) on 8 trn2 cores.

Sharding: core c -> batch b = c // 4, head-group g = c % 4 (4 heads each).
Data-parallel over B, tensor-parallel (Megatron) over heads for the
qkv / proj linears. Each core computes its head-group's attention and a
partial output projection; the host sums the 4 partials per batch and
adds the proj bias.

Structure (v1, from the ~161us baseline):
  * Inputs batched into ~11 DMA descriptors (descriptor issue on the Sync
    engine costs ~600ns each; the old ~30 descriptors serialized the
    startup). x is loaded ONCE in bf16; the fp8 copy for the q/k path is
    cast on-chip by the DVE (saves 2.1MB of HBM traffic on the critical
    startup path).
  * q/k projection (fp8 DoubleRow, weights pre-scaled x64 on host) runs
    kp-OUTER across 8 concurrent PSUM accumulation groups so the PE
    tracks the x DMA stream as chunk-pairs land.
  * v projection: first 4 row-chunks serial (k-inner), rest moved into
    the attention foreign queue.
  * Attention as 2 head-PAIRS, pt/et tiles laid out [128, 2, 512]
    (h0/h1 blocks); ONE exp ACTIVATE covers both heads via a 3D AP.
  * Causal mask: DVE multiply of the exp output's 128-wide diag strip by
    a constant 0/1 triangular [128, 2, 128] mask - off the PE entirely
    (the old identity-matmul mask cost ~417ns of LDW+MM per diag unit),
    hidden in the exp->AV lag slack.
  * Softmax denominator from a ones-column in v_aug (row 64 of the AV
    accumulator); reciprocal runs as [128,8] (DVE reciprocal costs
    ~6.5ns per free-column), reshaped via SBUF->SBUF DMA; broadcast
    across the 64 dim partitions via a DRAM round trip with
    partition-stride-0 read. The final block broadcasts on-chip (K=1
    matmul).
  * Stage B group B (q/k heads 2,3), v chunks 4-15 and the output
    projection are issued as closures interleaved one-per-unit into the
    attention loop to fill the ACT-bound PE gaps; ~6 closures are held
    back for the final normalize window so the PE (and its HAM clock)
    stays busy while the last block's reciprocal chain runs.
"""

import os
import sys
import types

for _p in ("/opt/trn_rl_repo", "/root/.axon_site", "/root/.axon_site/_ro/trn_rl_repo"):
    if os.path.isdir(_p) and _p not in sys.path:
        sys.path.append(_p)

import numpy as np
import ml_dtypes

import concourse.bacc as bacc
import concourse.bass as bass
import concourse.mybir as mybir
import concourse.tile as tile
from concourse.bass_utils import run_bass_kernel_spmd

# ── problem constants (hardcoded; spec.json not available at grade time) ──
B, T, C = 2, 2048, 1024
H, D = 16, 64
N_CORES = 8
HPG = 4                 # heads per group (per core)
CG = HPG * D            # 256 channels per head-group
NT = T // 512           # 4 query chunks of 512
KC = C // 128           # 8 contraction tiles for C
KP = KC // 2            # 4 fp8 DoubleRow K-pair chunks
VW = HPG * 65           # v tile width: 4x(64 dims + ones col) = 260
SC = 64.0               # fp8 weight pre-scale (power of two)

F32 = mybir.dt.float32
BF16 = mybir.dt.bfloat16
F8 = mybir.dt.float8e4
EXP = mybir.ActivationFunctionType.Exp
DR = mybir.MatmulPerfMode.DoubleRow
MULT = mybir.AluOpType.mult
ADD = mybir.AluOpType.add
NPBF = ml_dtypes.bfloat16
NPF8 = ml_dtypes.float8_e4m3fn

_trace_flag = [False]   # test.py can flip this to capture a profile
_last_results = [None]


def _ensure_ntff_hook():
    """Install the NTFF profile hook shim (container's antenv lacks it)."""
    if "antenv.axon_hooks" in sys.modules:
        return
    try:
        from trn_agent_boot.trn_boot import _ntff_profile_via_ctypes
    except Exception:
        return
    mod = types.ModuleType("antenv.axon_hooks")
    hook = [None]
    mod.set_axon_ntff_profile_hook = lambda h: hook.__setitem__(0, h)
    mod.get_axon_ntff_profile_hook = lambda: hook[0]
    sys.modules["antenv.axon_hooks"] = mod
    so = "/opt/axon/libaxon_pjrt.so"
    if os.path.exists(so):
        mod.set_axon_ntff_profile_hook(_ntff_profile_via_ctypes(so))


def build_nc():
    nc = bacc.Bacc("TRN2", target_bir_lowering=False, debug=False,
                   num_devices=N_CORES)

    # fp8 q/k weights, host-packed as [p, kp, s, f] -> [128, KP*2*2CG]
    wqk8_d = nc.dram_tensor("wqk8", [128, KP * 2 * 2 * CG], F8,
                            kind="ExternalInput").ap()
    # fp8 seed for the kp=3 chunk of x: the first DoubleRow matmul does not
    # wait for a bf16 chunk + cast, and the LAST-arriving bf16 chunk (xtp3)
    # is not on stage A's critical path; kp 0-2 are cast from xtp on-chip.
    xt8s_d = nc.dram_tensor("xt8s", [128, 2 * 2 * T], F8,
                            kind="ExternalInput").ap()
    xt_d = nc.dram_tensor("xt", [C, T], BF16, kind="ExternalInput").ap()
    wv_d = nc.dram_tensor("wv", [C, VW], BF16, kind="ExternalInput").ap()
    bqk_d = nc.dram_tensor("bqk", [2 * CG, 1], F32, kind="ExternalInput").ap()
    bvf_d = nc.dram_tensor("bvf", [128, VW], BF16, kind="ExternalInput").ap()
    wp_d = nc.dram_tensor("wp", [CG, C], BF16, kind="ExternalInput").ap()
    # tri2[k, s, m] = 1.0 where k <= m else 0 (causal diag strip mask)
    tri2_d = nc.dram_tensor("tri2", [128, 256], BF16, kind="ExternalInput").ap()
    ones16_d = nc.dram_tensor("ones16", [1, 64], BF16, kind="ExternalInput").ap()
    yt_d = nc.dram_tensor("yt", [C, T], BF16, kind="ExternalOutput").ap()
    rec_d = nc.dram_tensor("rec_scratch", [32, 512], F32).ap()

    with tile.TileContext(nc) as tc:
        with tc.tile_pool(name="const", bufs=1) as cp:
            # ── persistent SBUF residents ──
            wqk8 = cp.tile([128, KP, 2, 2 * CG], F8, tag="wqk8")
            xtp = [cp.tile([128, 2, T], BF16, tag=f"xtp{kp}", name=f"xtp{kp}")
                   for kp in range(KP)]
            xt8 = [cp.tile([128, 2, T], F8, tag=f"xt8{kp}", name=f"xt8{kp}")
                   for kp in range(KP)]
            warm = cp.tile([128, 128], BF16, tag="warm")
            wv = cp.tile([128, KC, VW], BF16, tag="wv")
            bqk = cp.tile([128, 4], F32, tag="bqk")
            bvf = cp.tile([128, VW], BF16, tag="bvf")
            wp = cp.tile([128, 2, C], BF16, tag="wp")
            tri2 = cp.tile([128, 2, 128], BF16, tag="tri2")
            ones16 = cp.tile([1, 64], BF16, tag="ones16")
            # qk[0]=q heads01, qk[1]=q heads23, qk[2]=k heads01, qk[3]=k heads23
            # (head pair laid out as rows 0-63 / 64-127 of the tile)
            qk = [cp.tile([128, T], BF16, tag=f"qk{m}", name=f"qk{m}")
                  for m in range(4)]
            v_sb = [cp.tile([128, VW], BF16, tag=f"v{m}", name=f"v{m}")
                    for m in range(T // 128)]
            outT = [cp.tile([128, T], BF16, tag=f"outT{k}", name=f"outT{k}")
                    for k in range(2)]

            # ── input DMA: few, large descriptors; x + q/k weights first ──
            nc.sync.dma_start(wqk8[:], wqk8_d[:, :])
            nc.sync.dma_start(xt8[2][:], xt8s_d[:, 0:2 * T])
            nc.sync.dma_start(xt8[3][:], xt8s_d[:, 2 * T:4 * T])
            for kp in range(2):
                nc.sync.dma_start(
                    xtp[kp][:],
                    bass.AP(xt_d.tensor, 256 * T * kp,
                            [[T, 128], [128 * T, 2], [1, T]]))
            nc.sync.dma_start(
                wv[:], bass.AP(wv_d.tensor, 0,
                               [[VW, 128], [128 * VW, KC], [1, VW]]))
            for kp in range(2, KP):
                nc.sync.dma_start(
                    xtp[kp][:],
                    bass.AP(xt_d.tensor, 256 * T * kp,
                            [[T, 128], [128 * T, 2], [1, T]]))
            nc.sync.dma_start(
                bqk[:], bass.AP(bqk_d.tensor, 0, [[1, 128], [128, 4]]))
            nc.sync.dma_start(bvf[:], bvf_d[:])
            nc.sync.dma_start(tri2[:], tri2_d[:])
            nc.sync.dma_start(
                wp[:], bass.AP(wp_d.tensor, 0,
                               [[C, 128], [128 * C, 2], [1, C]]))
            nc.sync.dma_start(ones16[:], ones16_d[:])

            # ── HAM warmup: ~3.5us of dummy matmuls with no DMA deps so the
            # PE clock gate opens before the first real (DMA-gated) matmul ──
            nc.vector.memset(warm[:], 0)
            with tc.tile_pool(name="psW", bufs=1, space="PSUM") as psW:
                pw = psW.tile([128, 512], F32, tag="pw")
                for w in range(34):
                    nc.tensor.matmul(pw[:, 0:128], warm[:], warm[:],
                                     start=True, stop=True)

            # on-chip bf16 -> fp8 casts for the q/k DoubleRow path (kp 0-2),
            # split per T-half so stage A's nt 0-1 matmuls start one half-
            # cast earlier
            for kp in range(2):
                for s in range(2):
                    for h in range(2):
                        nc.vector.tensor_copy(
                            xt8[kp][:, s, 1024 * h:1024 * (h + 1)],
                            xtp[kp][:, s, 1024 * h:1024 * (h + 1)])

            # ── stage A: q,k for heads 0,1 (mf 0 and 2). kp-OUTER across 8
            # concurrent PSUM groups so the PE tracks the x DMA stream; the
            # fp8 seed chunk (kp=3) runs first with no cast dependency. ──
            with tc.tile_pool(name="psAC", bufs=1, space="PSUM") as psA:
                pA = {}
                for mf in (0, 2):
                    for nt in range(NT):
                        pA[(mf, nt)] = psA.tile([128, 512], F32,
                                                tag=f"pA{mf}_{nt}",
                                                name=f"pA{mf}_{nt}")
                for kpi, kp in enumerate((2, 3, 0, 1)):
                    for nt in range(NT):
                        for mf in (0, 2):
                            nc.tensor.matmul(
                                pA[(mf, nt)][:],
                                wqk8[:, kp, :, 128 * mf:128 * (mf + 1)],
                                xt8[kp][:, :, 512 * nt:512 * (nt + 1)],
                                start=(kpi == 0), stop=(kpi == KP - 1),
                                perf_mode=DR)

                # ── bias-adds + stage C part 1 (v chunks 0-3), interleaved
                # so attention's j=0 gates (qk nt=0, v_sb[0..3]) clear as
                # early as possible. psv(mt) reuses pA(0,mt)'s PSUM bank,
                # whose bias-add runs just before it. The late k-head
                # bias-adds run on the (still idle) ACT engine so they do
                # not queue ahead of the v stt ops on the DVE. ──
                def bias(mf, nt, on_act=False):
                    if on_act:
                        nc.scalar.activation(
                            qk[mf][:, 512 * nt:512 * (nt + 1)],
                            pA[(mf, nt)][:],
                            mybir.ActivationFunctionType.Identity,
                            bias=bqk[:, mf:mf + 1], scale=1.0 / SC)
                    else:
                        nc.vector.tensor_scalar(
                            qk[mf][:, 512 * nt:512 * (nt + 1)],
                            pA[(mf, nt)][:], 1.0 / SC, bqk[:, mf:mf + 1],
                            op0=MULT, op1=ADD)

                bias(0, 0)
                bias(2, 0)
                for mt in range(4):
                    ps = psA.tile([128, 512], F32, tag=f"pA0_{mt}",
                                  name=f"psv{mt}")
                    for k in range(KC):
                        nc.tensor.matmul(
                            ps[:, 0:VW],
                            xtp[k // 2][:, k % 2, 128 * mt:128 * (mt + 1)],
                            wv[:, k, :], start=(k == 0), stop=(k == KC - 1))
                    nc.vector.scalar_tensor_tensor(
                        v_sb[mt][:], ps[:, 0:VW], 1.0, bvf[:],
                        op0=MULT, op1=ADD)
                    if mt < 3:
                        bias(0, mt + 1)
                for nt in range(1, NT):
                    bias(2, nt, on_act=True)

            # ── stage D: attention, two head-pairs. j outer, key chunk i
            # inner. Foreign PE work (stage B group B = q,k heads 2,3;
            # stage C tails; stage E output projection) interleaved one
            # closure per unit. ──
            with (
                tc.tile_pool(name="ptp", bufs=2, space="PSUM") as ptp,
                tc.tile_pool(name="avpp", bufs=1, space="PSUM") as avpp,
                tc.tile_pool(name="fxp", bufs=2, space="PSUM") as fxp,
                tc.tile_pool(name="etp", bufs=4) as etp,
                tc.tile_pool(name="rawp", bufs=2) as rawp,
                tc.tile_pool(name="recp", bufs=2) as recp,
                tc.tile_pool(name="bcp", bufs=2) as bcp,
                tc.tile_pool(name="otp", bufs=4) as otp,
            ):
                foreign = []

                # stage C tail closures: v chunks 4-15, two closures of 4
                # K-steps each.
                def mk_c(mt):
                    holder = {}
                    cls = []
                    for half in range(2):
                        def ccl(mt=mt, half=half, holder=holder):
                            if half == 0:
                                holder["ps"] = fxp.tile(
                                    [128, 512], F32, tag="fx",
                                    name=f"pc{mt}")
                            ps = holder["ps"]
                            for k in range(4 * half, 4 * half + 4):
                                nc.tensor.matmul(
                                    ps[:, 0:VW],
                                    xtp[k // 2][:, k % 2,
                                                128 * mt:128 * (mt + 1)],
                                    wv[:, k, :], start=(k == 0),
                                    stop=(k == KC - 1))
                            if half == 1:
                                nc.vector.scalar_tensor_tensor(
                                    v_sb[mt][:], ps[:, 0:VW], 1.0, bvf[:],
                                    op0=MULT, op1=ADD)
                        cls.append(ccl)
                    return cls

                for mt in range(4, 12):
                    foreign.extend(mk_c(mt))

                # stage B group B closures: mf 1 (q23) and 3 (k23), two
                # closures of 2 DoubleRow K-pair steps each.
                for mf in (1, 3):
                    for nt in range(NT):
                        holder = {}
                        for half in range(2):
                            def bgc(mf=mf, nt=nt, half=half, holder=holder):
                                if half == 0:
                                    holder["ps"] = fxp.tile(
                                        [128, 512], F32, tag="fx",
                                        name=f"bg{mf}_{nt}")
                                ps = holder["ps"]
                                for kp in (2 * half, 2 * half + 1):
                                    nc.tensor.matmul(
                                        ps[:],
                                        wqk8[:, kp, :,
                                             128 * mf:128 * (mf + 1)],
                                        xt8[kp][:, :,
                                                512 * nt:512 * (nt + 1)],
                                        start=(kp == 0), stop=(kp == KP - 1),
                                        perf_mode=DR)
                                if half == 1:
                                    nc.vector.tensor_scalar(
                                        qk[mf][:, 512 * nt:512 * (nt + 1)],
                                        ps[:], 1.0 / SC, bqk[:, mf:mf + 1],
                                        op0=MULT, op1=ADD)
                            foreign.append(bgc)
                for mt in range(12, T // 128):
                    foreign.extend(mk_c(mt))

                def mk_e(mo, nt):
                    def ecl():
                        ps = fxp.tile([128, 512], F32, tag="fx",
                                      name=f"pe{mo}_{nt}")
                        for k in range(2):
                            nc.tensor.matmul(
                                ps[:], wp[:, k, 128 * mo:128 * (mo + 1)],
                                outT[k][:, 512 * nt:512 * (nt + 1)],
                                start=(k == 0), stop=(k == 1))
                        ot = otp.tile([128, 512], BF16, tag="ot",
                                      name=f"ot{mo}_{nt}")
                        if nt == 3 and mo % 2 == 0:
                            # tail chunk: ACT is idle by then; alternate
                            # with DVE so neither queue paces the drain
                            nc.scalar.copy(ot[:], ps[:])
                        else:
                            nc.vector.tensor_copy(ot[:], ps[:])
                        nc.sync.dma_start(
                            yt_d[128 * mo:128 * (mo + 1),
                                 512 * nt:512 * (nt + 1)], ot[:])
                    return ecl

                def do_av(p, j, i, c0, et, avp0, avp1):
                    last = (i == 4 * j + 3)
                    nc.tensor.matmul(
                        avp0[:, c0:512],
                        v_sb[i][:, 130 * p:130 * p + 65],
                        et[:, 0, c0:512], start=(i == 0), stop=last)
                    nc.tensor.matmul(
                        avp1[:, c0:512],
                        v_sb[i][:, 130 * p + 65:130 * p + 130],
                        et[:, 1, c0:512],
                        start=(i == 0), stop=last)

                def normalize(p, j, avp0, avp1):
                    # DVE reciprocal costs ~6.5ns/free-column no matter
                    # the partition count, so reshape the 2x512 denoms
                    # to [128,8] via SBUF->SBUF DMA (26ns recip). Raw
                    # copies come first so both avp banks free
                    # immediately for the next j's accumulation.
                    u2 = 8 * p + 2 * j
                    raws = []
                    den2 = recp.tile([128, 8], F32, tag="den2",
                                     name=f"den2_{p}_{j}")
                    for hh in range(2):
                        avp = avp0 if hh == 0 else avp1
                        raw = rawp.tile([65, 512], F32, tag=f"raw{hh}",
                                        name=f"raw{p}_{hh}_{j}")
                        nc.vector.tensor_copy(raw[:], avp[:])
                        nc.sync.dma_start(den2[64 * hh:64 * hh + 64, :],
                                          raw[64:65, :])
                        raws.append(raw)
                    rec2 = recp.tile([128, 8], F32, tag="rec2",
                                     name=f"rec2_{p}_{j}")
                    nc.vector.reciprocal(rec2[:], den2[:])
                    nc.sync.dma_start(
                        bass.AP(rec_d.tensor, u2 * 512,
                                [[8, 128], [1, 8]]), rec2[:])
                    for hh in range(2):
                        bc = bcp.tile([64, 512], F32, tag=f"bc{hh}",
                                      name=f"bc{p}_{hh}_{j}")
                        nc.sync.dma_start(
                            bc[:], bass.AP(rec_d.tensor,
                                           (u2 + hh) * 512,
                                           [[0, 64], [1, 512]]))
                        nc.vector.tensor_mul(
                            outT[p][64 * hh:64 * hh + 64,
                                    512 * j:512 * (j + 1)],
                            raws[hh][0:64, :], bc[:])
                    if p == 1:
                        for mo in range(8):
                            foreign.append(mk_e(mo, j))

                def norm_final_head(p, j, avp0, avp1):
                    # final block, part 1: everything up to the rec_row
                    # reshape DMA — no PE instructions, so the reserved
                    # foreign closures drained right after this fill the
                    # PE while the DMA latency plays out. One raw copy on
                    # ACT (idle by now) so the pair runs in parallel.
                    raws = []
                    den2 = recp.tile([128, 8], F32, tag="den2",
                                     name="den2_f")
                    for hh in range(2):
                        avp = avp0 if hh == 0 else avp1
                        raw = rawp.tile([65, 512], F32, tag=f"raw{hh}",
                                        name=f"rawf_{hh}")
                        if hh == 1:
                            nc.scalar.copy(raw[:], avp[:])
                        else:
                            nc.vector.tensor_copy(raw[:], avp[:])
                        nc.sync.dma_start(den2[64 * hh:64 * hh + 64, :],
                                          raw[64:65, :])
                        raws.append(raw)
                    rec2 = recp.tile([128, 8], F32, tag="rec2",
                                     name="rec2_f")
                    nc.vector.reciprocal(rec2[:], den2[:])
                    rec2b = recp.tile([128, 8], BF16, tag="rec2b",
                                      name="rec2b")
                    nc.vector.tensor_copy(rec2b[:], rec2[:])
                    rec_row = recp.tile([1, 1024], BF16,
                                        tag="recrow", name="recrow")
                    nc.sync.dma_start(rec_row[:], rec2b[:])
                    return p, j, raws, rec_row

                def norm_final_tail(p, j, raws, rec_row):
                    # final block, part 2: on-chip K=1 PE broadcast of the
                    # reciprocals + the normalize multiplies, then the j=3
                    # output-projection closures.
                    for hh in range(2):
                        bcps = fxp.tile([128, 512], F32, tag="fx",
                                        name=f"bcps{hh}")
                        nc.tensor.matmul(
                            bcps[0:64, :], ones16[:],
                            rec_row[0:1, 512 * hh:512 * (hh + 1)],
                            start=True, stop=True)
                        nc.vector.tensor_mul(
                            outT[p][64 * hh:64 * hh + 64,
                                    512 * j:512 * (j + 1)],
                            raws[hh][0:64, :], bcps[0:64, :])
                    for mo in range(8):
                        foreign.append(mk_e(mo, j))

                # flattened unit stream, software-pipelined ACROSS j (and
                # pair) boundaries: the next block's attT+exp issue before
                # the previous block's last AV flush and normalize, so the
                # ACT engine never waits at a block boundary.
                units = [(p, j, i) for p in range(2) for j in range(NT)
                         for i in range(4 * j + 4)]
                n_units = len(units)
                pending = None
                prev_norm = None
                avp0 = avp1 = None
                for uidx, (p, j, i) in enumerate(units):
                    qt = qk[p]
                    kt = qk[2 + p]
                    if i == 0:
                        avp0 = avpp.tile([65, 512], F32, tag="avp0",
                                         name=f"avp0_{p}_{j}")
                        avp1 = avpp.tile([65, 512], F32, tag="avp1",
                                         name=f"avp1_{p}_{j}")
                    diag = i >= 4 * j
                    c0 = 128 * (i - 4 * j) if diag else 0
                    pt = ptp.tile([128, 2, 512], F32, tag="pt",
                                  name=f"pt{p}_{j}_{i}")
                    # attT: both heads concurrently (row halves of the PE
                    # array) into the two 512-col blocks of one 2-bank
                    # PSUM tile; ONE exp ACTIVATE covers both via a 3D AP.
                    nc.tensor.matmul(
                        pt[:, 0, c0:512],
                        kt[0:64, 128 * i:128 * (i + 1)],
                        qt[0:64, 512 * j + c0:512 * (j + 1)],
                        start=True, stop=True)
                    nc.tensor.matmul(
                        pt[:, 1, c0:512],
                        kt[64:128, 128 * i:128 * (i + 1)],
                        qt[64:128, 512 * j + c0:512 * (j + 1)],
                        start=True, stop=True)
                    et = etp.tile([128, 2, 512], BF16, tag="et",
                                  name=f"et{p}_{j}_{i}")
                    nc.scalar.activation(et[:, :, c0:512],
                                         pt[:, :, c0:512], EXP)
                    # foreign closure BEFORE the lagged AV: gives the
                    # previous unit's exp a few hundred ns to finish so the
                    # AV never stalls the PE queue on the ACT engine.
                    # In the second half, hold ~8 closures back so the PE
                    # stays busy (and HAM stays warm) through the final
                    # normalize chain.
                    if foreign and (uidx < 44 or len(foreign) > 12):
                        foreign.pop(0)()
                    if pending is not None:
                        do_av(*pending)
                    pending = (p, j, i, c0, et, avp0, avp1)
                    if prev_norm is not None:
                        normalize(*prev_norm)
                        prev_norm = None
                    if i == 4 * j + 3:
                        prev_norm = (p, j, avp0, avp1)
                    if diag:
                        # causal mask: zero the upper triangle of the
                        # 128-wide diag strip (both heads in one DVE op,
                        # hidden in the exp->AV lag slack). Issued LAST in
                        # the unit so it does not block the strict-FIFO
                        # DVE queue while its exp is still running.
                        nc.vector.tensor_mul(et[:, :, c0:c0 + 128],
                                             et[:, :, c0:c0 + 128],
                                             tri2[:])
                do_av(*pending)
                # final block: start the reciprocal chain (DVE/ACT/DMA
                # only), fill the PE with the reserved closures while its
                # DMA latency plays out, then the broadcast+muls and the
                # j=3 output projection.
                saved = norm_final_head(*prev_norm)
                while foreign:
                    foreign.pop(0)()
                norm_final_tail(*saved)
                while foreign:
                    foreign.pop(0)()

    nc.compile()
    return nc


def _pack8(a):
    """[C, F] f32 -> [128, KP*2*F] fp8 host layout [p, kp, s, f]."""
    Cd, F = a.shape
    return np.ascontiguousarray(
        a.reshape(KP, 2, 128, F).transpose(2, 0, 1, 3).reshape(128, -1)
    ).astype(NPF8)


def _shard_inputs(x, w_qkv, b_qkv, w_proj):
    scale = 1.0 / np.sqrt(D)   # 0.125, exact power of two
    in_maps = []
    k_idx = np.arange(128)[:, None]
    m_idx = np.arange(128)[None, :]
    tri = (k_idx <= m_idx).astype(np.float32)
    tri2 = np.concatenate([tri, tri], axis=1).astype(NPBF)
    for core in range(N_CORES):
        b, g = divmod(core, HPG)
        qs = slice(CG * g, CG * (g + 1))
        ks = slice(C + CG * g, C + CG * (g + 1))
        vs = slice(2 * C + CG * g, 2 * C + CG * (g + 1))
        wqk = np.concatenate([w_qkv[qs] * scale, w_qkv[ks]], axis=0).T
        bqk = np.concatenate([b_qkv[qs] * scale, b_qkv[ks]])[:, None]
        wv_base = w_qkv[vs].T          # [C, 256]
        wv = np.zeros((C, VW), np.float32)
        bv = np.zeros((1, VW), np.float32)
        for h in range(HPG):
            wv[:, 65 * h:65 * h + 64] = wv_base[:, 64 * h:64 * h + 64]
            bv[0, 65 * h:65 * h + 64] = b_qkv[vs][64 * h:64 * h + 64]
            bv[0, 65 * h + 64] = 1.0
        xb = np.ascontiguousarray(x[b].T)          # [C, T]
        xt8s = np.ascontiguousarray(
            xb[512:1024].reshape(2, 2, 128, T).transpose(2, 0, 1, 3)
            .reshape(128, 4 * T)).astype(NPF8)
        in_maps.append({
            "wqk8": _pack8(wqk * SC),
            "xt8s": xt8s,
            "xt": xb.astype(NPBF),
            "wv": wv.astype(NPBF),
            "bqk": np.ascontiguousarray(bqk, np.float32),
            "bvf": np.broadcast_to(bv, (128, VW)).astype(NPBF),
            "wp": np.ascontiguousarray(
                w_proj[:, CG * g:CG * (g + 1)].T).astype(NPBF),
            "tri2": tri2,
            "ones16": np.ones((1, 64), NPBF),
        })
    return in_maps


def kernel(x, w_qkv, b_qkv, w_proj, b_proj):
    x = np.asarray(x, np.float32)
    w_qkv = np.asarray(w_qkv, np.float32)
    b_qkv = np.asarray(b_qkv, np.float32)
    w_proj = np.asarray(w_proj, np.float32)
    b_proj = np.asarray(b_proj, np.float32)

    nc = build_nc()
    in_maps = _shard_inputs(x, w_qkv, b_qkv, w_proj)
    if _trace_flag[0]:
        _ensure_ntff_hook()
    res = run_bass_kernel_spmd(nc, in_maps, core_ids=list(range(N_CORES)),
                               trace=_trace_flag[0])
    _last_results[0] = res

    y = np.empty((B, T, C), np.float32)
    for b in range(B):
        acc = np.zeros((C, T), np.float32)
        for g in range(HPG):
            acc += np.asarray(res.results[HPG * b + g]["yt"], np.float32)
        y[b] = acc.T + b_proj[None, :]
    return y
